# revision 1
# baseline (speedup 1.0000x reference)
"""Trainium2 Bass kernel for nn_Block_40742059770386 (dense_cnn).

Per-sample adaptively-mixed, style-modulated, demodulated 3x3 conv
(StyleGAN2-style) + channel RMS norm + SiLU.

Sharding: data-parallel over batch. B=16 samples -> 8 cores x 2 samples.
The small kernel bank (2 x 256 x 256 x 3 x 3) and gamma are replicated.

Key reformulation (avoids materializing per-sample demodulated weights):
    w       = (a0*W0 + a1*W1) * (mod+1)[i] * d[o]
    y_conv  = conv(x_mod, a0*W0 + a1*W1),  x_mod = x * (mod+1)[i]
    d[o]    = rsqrt(max(sum_i m2[i] * sum_kk wmix[o,i,kk]^2, eps)),
              m2[i] = (mod[i]+1)^2     (tiny fp32 matvec on TensorE)
    nsum[p] = sum_o (d[o]*y_conv[o,p])^2 = sum_o d[o]^2 * y_conv[o,p]^2
              (matmul with lhsT = d^2 column; row per pixel-tile in PSUM)
    out     = silu(y_conv * inv_norm[p] * (d[o]*gamma[o]*sqrt(256)))

Engine notes:
  - conv = implicit GEMM over zero-padded x [128, 66, 66] with shifted APs;
    fp32r matmuls (full PE rate at N=512); every fp32r operand is produced
    by a compute op with float32r output dtype (walrus requires rounding).
  - ACT functions limited to {Square, Sqrt, Sigmoid} with Sqrt batched
    once per sample to minimize the 1283ns activation-table reloads.
  - demod rsqrt done DVE-only via bit-trick seed + 2 Newton steps.
  - y_conv staged via DRAM (PSUM->DRAM->SBUF) so PSUM banks free per
    pixel-tile while the channel-norm is batched per sample.
"""

import os
import numpy as np

import concourse.bass as bass
import concourse.bacc as bacc
import concourse.mybir as mybir
import concourse.tile as tile
from contextlib import ExitStack
from concourse.bass_utils import run_bass_kernel_spmd
from concourse import bass_isa

# ---- problem constants (hardcoded; kernel.py must be self-contained) ----
B, C_IN, C_OUT, H, W, K, NK = 16, 256, 256, 64, 64, 3, 2
N_CORES = 8
S = B // N_CORES            # samples per core
PB = 128                    # partitions per block
IB = C_IN // PB             # input channel blocks
OB = C_OUT // PB            # output channel blocks
HW = H * W                  # 4096
PADH, PADW = H + 2, W + 2   # 66, 66
PT = 512                    # pixels per tile (one PSUM bank of fp32)
ROWS_PT = PT // W           # 8 rows per pixel tile
NPT = HW // PT              # 8 pixel tiles
KK = K * K                  # 9
NVEC = 2 + IB + 3 * IB      # packed per-sample vector columns
EPS = 1e-8

F32 = mybir.dt.float32
F32R = mybir.dt.float32r
BF16 = mybir.dt.bfloat16
I32 = mybir.dt.int32

# "f32r": full-rate near-fp32 matmuls; "bf16": bf16 matmuls
MM_MODE = os.environ.get("KERNEL_MM_MODE", "f32r")

AF = mybir.ActivationFunctionType
ALU = mybir.AluOpType
MAGIC = 0x5F3759DF
# CoreSim does not implement Silu; decompose for sim-only runs
SIM_SILU = os.environ.get("KERNEL_SIM_SILU", "0") == "1"
NORM_ITERS = int(os.environ.get("KERNEL_NORM_ITERS", "1"))


def _newton_rsqrt_steps(nc, pool, r, x, shape, tag, iters):
    """Refine r ~ rsqrt(x): r' = r * (1.5 - 0.5 * x * r^2). Returns tile."""
    xh = pool.tile(shape, F32, tag=f"{tag}_xh", name=f"{tag}_xh")
    nc.vector.tensor_scalar_mul(out=xh, in0=x, scalar1=0.5)
    for it in range(iters):
        t = pool.tile(shape, F32, tag=f"{tag}_t{it}", name=f"{tag}_t{it}")
        nc.vector.tensor_mul(out=t, in0=r, in1=r)
        nc.vector.tensor_mul(out=t, in0=t, in1=xh)
        nc.vector.tensor_scalar(
            out=t, in0=t, scalar1=-1.0, scalar2=1.5, op0=ALU.mult, op1=ALU.add
        )
        r2 = pool.tile(shape, F32, tag=f"{tag}_r{it}", name=f"{tag}_r{it}")
        nc.vector.tensor_mul(out=r2, in0=r, in1=t)
        r = r2
    return r


def _rsqrt_dve(nc, pool, src_ap, clamp, shape, tag, iters=2):
    """rsqrt(max(src, clamp)) entirely on DVE: bit-trick seed + Newton."""
    x = pool.tile(shape, F32, tag=f"{tag}_x", name=f"{tag}_x")
    nc.vector.tensor_scalar_max(out=x, in0=src_ap, scalar1=float(clamp))
    seed = pool.tile(shape, I32, tag=f"{tag}_s", name=f"{tag}_s")
    nc.vector.tensor_scalar(
        out=seed, in0=x.bitcast(I32), scalar1=1, scalar2=None,
        op0=ALU.logical_shift_right,
    )                                   # bits >> 1
    nc.vector.tensor_scalar(
        out=seed, in0=seed, scalar1=-1, scalar2=MAGIC,
        op0=ALU.mult, op1=ALU.add,
    )                                   # MAGIC - (bits >> 1)
    return _newton_rsqrt_steps(nc, pool, seed.bitcast(F32), x, shape, tag, iters=iters)


def _rsqrt_act(nc, pool, src_ap, clamp, shape, tag):
    """rsqrt(max(src, clamp)) via ACT Sqrt + DVE recip + 1 Newton step."""
    x = pool.tile(shape, F32, tag=f"{tag}_x", name=f"{tag}_x")
    nc.vector.tensor_scalar_max(out=x, in0=src_ap, scalar1=float(clamp))
    r = pool.tile(shape, F32, tag=f"{tag}_r0", name=f"{tag}_r0")
    nc.scalar.activation(out=r, in_=x, func=AF.Sqrt)
    nc.vector.reciprocal(out=r, in_=r)
    return _newton_rsqrt_steps(nc, pool, r, x, shape, tag, iters=1)


def build_program(mm_mode=MM_MODE):
    nc = bacc.Bacc(trn_type="TRN2", debug=False)

    x_d = nc.declare_dram_parameter("x", [S, IB, PB, HW], F32, isOutput=False)
    wt_d = nc.declare_dram_parameter("wT", [NK, IB, PB, C_OUT, KK], F32, isOutput=False)
    vecs_d = nc.declare_dram_parameter("vecs", [S, PB, NVEC], F32, isOutput=False)
    smat_d = nc.declare_dram_parameter("smat", [3, IB, PB, C_OUT], F32, isOutput=False)
    g16_d = nc.declare_dram_parameter("g16", [OB, PB, 1], F32, isOutput=False)
    y_d = nc.declare_dram_parameter("y", [S, OB, PB, HW], F32, isOutput=True)

    mm_dt = {"f32r": F32R, "bf16": BF16}[mm_mode]

    with ExitStack() as ctx:
        tc = ctx.enter_context(tile.TileContext(nc))
        const = ctx.enter_context(tc.tile_pool(name="const", bufs=1))
        wpool = ctx.enter_context(tc.tile_pool(name="wmix", bufs=3))
        xfp = ctx.enter_context(tc.tile_pool(name="xf32", bufs=2))
        xrp = ctx.enter_context(tc.tile_pool(name="xpad", bufs=3))
        small = ctx.enter_context(tc.tile_pool(name="small", bufs=4))
        rows = ctx.enter_context(tc.tile_pool(name="rows", bufs=1))
        sq_p = ctx.enter_context(tc.tile_pool(name="ycsq", bufs=3))
        ycp = ctx.enter_context(tc.tile_pool(name="ycpool", bufs=13))
        outp = ctx.enter_context(tc.tile_pool(name="outs", bufs=2))
        bcastp = ctx.enter_context(tc.tile_pool(name="bcast", bufs=3))
        dramp = ctx.enter_context(tc.tile_pool(name="dram", bufs=2, space="DRAM"))
        pconv = ctx.enter_context(tc.tile_pool(name="pconv", bufs=5, space="PSUM"))
        pnorm = ctx.enter_context(tc.tile_pool(name="pnorm", bufs=2, space="PSUM"))
        pdsq = ctx.enter_context(tc.tile_pool(name="pdsq", bufs=1, space="PSUM"))

        # ---- resident constants ----
        wbank = [
            [const.tile([PB, C_OUT, KK], F32, tag=f"wb{n}{ib}", name=f"wb{n}{ib}")
             for ib in range(IB)]
            for n in range(NK)
        ]
        g16sb = [const.tile([PB, 1], F32, tag=f"g16_{ob}", name=f"g16_{ob}")
                 for ob in range(OB)]
        ones_row = const.tile([1, PB], F32, tag="ones_row", name="ones_row")
        nc.vector.memset(ones_row, 1.0)
        smat = [
            [const.tile([PB, C_OUT], F32, tag=f"sm{k}{ib}", name=f"sm{k}{ib}")
             for ib in range(IB)]
            for k in range(3)
        ]

        for s in range(S):
            # ---- packed per-sample vectors: one DMA ----
            vec = small.tile([PB, NVEC], F32, tag="vec", name="vec")
            nc.sync.dma_start(out=vec, in_=vecs_d[s])
            acol = [vec[:, n:n + 1] for n in range(NK)]
            mpc = [vec[:, 2 + ib:3 + ib] for ib in range(IB)]
            m2k = [[vec[:, 4 + 3 * ib + k:5 + 3 * ib + k] for k in range(3)]
                   for ib in range(IB)]
            if s == 0:
                nc.sync.dma_start(out=g16sb[0], in_=g16_d[0])
                nc.sync.dma_start(out=g16sb[1], in_=g16_d[1])


            # ---- mix kernel bank + x-prep (interleaved for fast conv start) ----
            dsq_ps = [pdsq.tile([PB, 1], F32, tag="dsq", name="dsq")
                      for _ in range(OB)]
            wmix = []
            xp = []
            xfs = []
            HH = H // 2
            # DMA emission order = model's serial-DMA order: weight-bank halves
            # first (o-half 0 for both banks of both ib), x row-halves woven in.
            QH = H // 4
            if s == 0:
                for n in range(NK):
                    for ib in range(IB):
                        nc.sync.dma_start(
                            out=wbank[n][ib][:, 0:PB, :], in_=wt_d[n, ib, :, 0:PB, :]
                        )
            for ib in range(IB):
                xf = xfp.tile([PB, HW], F32, tag="xf32", name="xf32")
                nc.sync.dma_start(out=xf[:, 0:QH * W], in_=x_d[s, ib, :, 0:QH * W])
                xfs.append(xf)
            for ib in range(IB):
                nc.sync.dma_start(
                    out=xfs[ib][:, QH * W:HH * W], in_=x_d[s, ib, :, QH * W:HH * W]
                )
            if s == 0:
                for n in range(NK):
                    for ib in range(IB):
                        nc.sync.dma_start(
                            out=wbank[n][ib][:, PB:C_OUT, :],
                            in_=wt_d[n, ib, :, PB:C_OUT, :],
                        )
            for ib in range(IB):
                nc.sync.dma_start(
                    out=xfs[ib][:, HH * W:HW], in_=x_d[s, ib, :, HH * W:HW]
                )
            if s == 0:
                for k in range(3):
                    for ib in range(IB):
                        nc.sync.dma_start(out=smat[k][ib], in_=smat_d[k, ib])
            for ib in range(IB):
                wm = wpool.tile([PB, C_OUT, KK], mm_dt, tag="wmix", name="wmix")
                for oh in range(OB):
                    osl = slice(oh * PB, (oh + 1) * PB)
                    nc.vector.tensor_scalar_mul(
                        out=wm[:, osl, :], in0=wbank[0][ib][:, osl, :], scalar1=acol[0]
                    )
                    nc.vector.scalar_tensor_tensor(
                        out=wm[:, osl, :], in0=wbank[1][ib][:, osl, :],
                        scalar=acol[1], in1=wm[:, osl, :],
                        op0=ALU.mult, op1=ALU.add,
                    )
                wmix.append(wm)
                xr = xrp.tile([PB, PADH, PADW], mm_dt, tag="xpad", name="xpad")
                bc = F32 if mm_dt == F32R else mm_dt
                nc.gpsimd.memset(xr[:, 0:1, :].bitcast(bc), 0.0)
                nc.gpsimd.memset(xr[:, PADH - 1:PADH, :].bitcast(bc), 0.0)
                nc.gpsimd.memset(xr[:, 1:H + 1, 0:1].bitcast(bc), 0.0)
                nc.gpsimd.memset(xr[:, 1:H + 1, PADW - 1:PADW].bitcast(bc), 0.0)
                for r0, r1 in ((0, QH), (QH, HH), (HH, H)):
                    nc.vector.tensor_scalar_mul(
                        out=xr[:, 1 + r0:1 + r1, 1:W + 1],
                        in0=xfs[ib][:, r0 * W:r1 * W].rearrange(
                            "p (h w) -> p h w", w=W
                        ),
                        scalar1=mpc[ib],
                    )
                xp.append(xr)
            # demod via precomputed bank Gram stats:
            #   sum_kk wmix^2 = a0^2*S00 + 2a0a1*S01 + a1^2*S11,
            # with the a-coefficients folded into host-packed m2 columns.
            for ob in range(OB):
                i_mv = 0
                for ib in range(IB):
                    for k in range(3):
                        nc.tensor.matmul(
                            dsq_ps[ob],
                            lhsT=smat[k][ib][:, ob * PB:(ob + 1) * PB],
                            rhs=m2k[ib][k],
                            start=(i_mv == 0),
                            stop=(i_mv == 3 * IB - 1),
                        )
                        i_mv += 1

            # d = rsqrt(max(dsq, EPS)) on DVE; d2 (matmul dtype); gd = d*g16
            dcol, gdcol = [], []
            for ob in range(OB):
                d = _rsqrt_dve(nc, small, dsq_ps[ob], EPS, [PB, 1], f"d{ob}")
                dcol.append(d)
                gd = small.tile([PB, 1], F32, tag=f"gd{ob}", name=f"gd{ob}")
                nc.vector.tensor_mul(out=gd, in0=d, in1=g16sb[ob])
                gdcol.append(gd)

            # ---- conv (implicit GEMM); norm batched per group ----
            GROUPS = [4, 2, 2]     # asymmetric: small last group shrinks the tail
            for g, GPT in enumerate(GROUPS):
                g0 = sum(GROUPS[:g])
                dgath = dramp.tile([GPT, PT], F32, tag="dgath", name="dgath")
                ycg = {}
                for lpt in range(GPT):
                    pt = g0 + lpt
                    sqs = []
                    for ob in range(OB):
                        ps = pconv.tile([PB, PT], F32, tag="conv", name="conv")
                        n_mm = IB * KK
                        i_mm = 0
                        for ib in range(IB):
                            for ki in range(K):
                                for kj in range(K):
                                    lhsT = wmix[ib][:, ob * PB:(ob + 1) * PB, ki * K + kj]
                                    rhs = xp[ib][
                                        :,
                                        pt * ROWS_PT + ki: pt * ROWS_PT + ki + ROWS_PT,
                                        kj: kj + W,
                                    ]
                                    nc.tensor.matmul(
                                        ps, lhsT=lhsT, rhs=rhs,
                                        start=(i_mm == 0), stop=(i_mm == n_mm - 1),
                                    )
                                    i_mm += 1
                        sq = sq_p.tile([PB, PT], F32, tag="ycsq", name="ycsq")
                        nc.scalar.activation(
                            out=sq, in_=ps, func=AF.Square, scale=dcol[ob]
                        )   # (d*y)^2 = d^2 y^2
                        yc = ycp.tile([PB, PT], BF16, tag="yc", name="yc")
                        nc.scalar.activation(out=yc, in_=ps, func=AF.Copy)
                        ycg[(lpt, ob)] = yc
                        # channel sum on the idle Pool engine (in-place all-reduce)
                        nc.gpsimd.partition_all_reduce(
                            sq[:], sq[:], PB, bass_isa.ReduceOp.add
                        )
                        sqs.append(sq)
                    nrow = rows.tile([1, PT], F32, tag="nrow", name="nrow")
                    nc.vector.tensor_add(out=nrow, in0=sqs[0][0:1, :], in1=sqs[1][0:1, :])
                    nc.sync.dma_start(out=dgath[lpt], in_=nrow)

                # norm chain for this group
                gath = rows.tile([GPT, PT], F32, tag="gath", name="gath")
                nc.sync.dma_start(out=gath, in_=dgath)
                invg = _rsqrt_dve(nc, rows, gath, 1e-24, [GPT, PT], "inv", iters=NORM_ITERS)
                dinv = dramp.tile([GPT, PT], F32, tag="dinv", name="dinv")
                nc.sync.dma_start(out=dinv, in_=invg)
                for lpt in range(GPT):
                    pt = g0 + lpt
                    invb = bcastp.tile([PB, PT], F32, tag="invb", name="invb")
                    nc.sync.dma_start(
                        out=invb, in_=dinv[lpt:lpt + 1, :].to_broadcast((PB, PT))
                    )
                    for ob in range(OB):
                        z = outp.tile([PB, PT], F32, tag="z", name="z")
                        nc.vector.scalar_tensor_tensor(
                            out=z, in0=ycg[(lpt, ob)], scalar=gdcol[ob], in1=invb,
                            op0=ALU.mult, op1=ALU.mult,
                        )
                        yo = outp.tile([PB, PT], F32, tag="yo", name="yo")
                        if SIM_SILU:
                            nc.scalar.activation(out=yo, in_=z, func=AF.Sigmoid)
                            nc.vector.tensor_mul(out=yo, in0=z, in1=yo)
                        else:
                            nc.scalar.activation(out=yo, in_=z, func=AF.Silu)
                        nc.gpsimd.dma_start(
                            out=y_d[s, ob, :, pt * PT:(pt + 1) * PT], in_=yo
                        )
    nc.finalize()
    return nc


_NC_CACHE = {}


def _get_program(mm_mode=MM_MODE):
    if mm_mode not in _NC_CACHE:
        _NC_CACHE[mm_mode] = build_program(mm_mode)
    return _NC_CACHE[mm_mode]


def _host_prep(x, mod, kernel_mod, weights, gamma):
    x = np.asarray(x, dtype=np.float32)
    mod = np.asarray(mod, dtype=np.float32)
    kernel_mod = np.asarray(kernel_mod, dtype=np.float32)
    weights = np.asarray(weights, dtype=np.float32)
    gamma = np.asarray(gamma, dtype=np.float32)

    # softmax over the (tiny) kernel bank dim
    e = np.exp(kernel_mod - kernel_mod.max(axis=-1, keepdims=True))
    attn = (e / e.sum(axis=-1, keepdims=True)).astype(np.float32)     # [B, NK]

    modp1 = mod + 1.0                                                 # [B, C_IN]
    m2 = modp1 * modp1

    # [NK, O, I, K, K] -> [NK, I, O, K*K] -> [NK, IB, PB, C_OUT, KK]
    wT = np.ascontiguousarray(
        weights.transpose(0, 2, 1, 3, 4).reshape(NK, IB, PB, C_OUT, KK)
    )
    # bank Gram stats over kk: S00, S01, S11 as [i, o], split by i-block
    wio = weights.transpose(0, 2, 1, 3, 4).reshape(NK, C_IN, C_OUT, KK)
    s00 = (wio[0] * wio[0]).sum(-1)
    s01 = (wio[0] * wio[1]).sum(-1)
    s11 = (wio[1] * wio[1]).sum(-1)
    smat = np.ascontiguousarray(
        np.stack([s00, s01, s11]).reshape(3, IB, PB, C_OUT).astype(np.float32)
    )
    g16 = np.ascontiguousarray(
        (gamma * np.sqrt(C_OUT)).astype(np.float32).reshape(OB, PB, 1)
    )

    in_maps = []
    for c in range(N_CORES):
        sl = slice(c * S, (c + 1) * S)
        vecs = np.empty((S, PB, NVEC), np.float32)
        for si in range(S):
            b = c * S + si
            a0, a1 = attn[b, 0], attn[b, 1]
            vecs[si, :, 0] = a0
            vecs[si, :, 1] = a1
            vecs[si, :, 2:2 + IB] = modp1[b].reshape(IB, PB).T
            m2b = m2[b].reshape(IB, PB)
            for ib in range(IB):
                vecs[si, :, 4 + 3 * ib] = m2b[ib] * (a0 * a0)
                vecs[si, :, 5 + 3 * ib] = m2b[ib] * (2.0 * a0 * a1)
                vecs[si, :, 6 + 3 * ib] = m2b[ib] * (a1 * a1)
        in_maps.append({
            "x": np.ascontiguousarray(x[sl].reshape(S, IB, PB, HW)),
            "wT": wT,
            "smat": smat,
            "vecs": vecs,
            "g16": g16,
        })
    return in_maps


def kernel(x, mod, kernel_mod, weights, gamma, _trace=False, _trace_kwargs=None):
    nc = _get_program()
    in_maps = _host_prep(x, mod, kernel_mod, weights, gamma)
    res = run_bass_kernel_spmd(
        nc, in_maps, list(range(N_CORES)),
        trace=_trace, **(_trace_kwargs or {}),
    )
    y = np.concatenate(
        [res.results[c]["y"].reshape(S, C_OUT, H, W) for c in range(N_CORES)],
        axis=0,
    ).astype(np.float32)
    if _trace:
        kernel.last_results = res
    return y


kernel.last_results = None



# revision 34
# speedup vs baseline: 1.5378x; 1.5378x over previous
"""Trainium2 Bass kernel for nn_Block_40742059770386 (dense_cnn).

Per-sample adaptively-mixed, style-modulated, demodulated 3x3 conv
(StyleGAN2-style) + channel RMS norm + SiLU.

Sharding: data-parallel over batch. B=16 samples -> 8 cores x 2 samples.
The kernel bank (2 x 256 x 256 x 3 x 3, shipped bf16) and gamma are
replicated.

Conv engine strategy (the PE-time floor):
  fp8e4m3 DoubleRow matmuls contract all 256 input channels per
  instruction at 0.5 cycles/row -- 4x the f32r row rate.  Raw fp8 on
  both operands misses the 2e-2 tolerance (~4e-2), so the conv runs a
  3-term hi/lo split at a shared PSUM scale S=128:

      S*w*x ~= w8@x8 + wlo16@x8 + w8_16@xlo8
      w8    = fp8(S*wm)            (wm = bf16-mixed modulated weights)
      wlo16 = fp8(S*wm - w8)       (weight residual, same S scale)
      w8_16 = fp8(S/16*wm)         (for the x-residual term)
      x8    = fp8(x), xlo8 = fp8(16*(x - x8))   (host-side dtype prep)

  27 DoubleRow matmuls per [128 out, 512 px] tile vs 18 f32r ones:
  1.33x less PE time at ~bf16 accuracy (measured 4.6e-3 end to end).

Modulation is folded into the mix coefficients (a_n * (mod+1)[ci] as
per-partition scalar columns), so x ships unmodulated and pre-padded;
no on-chip modulate/pad pass at all.

Epilogue: Pool drains PSUM -> f16 (427ns/tile), everything else in f16
on DVE at 2x/4x rates; channel-sum via gpsimd partition_all_reduce
(fp32 internal), rsqrt = ACT Sqrt + DVE reciprocal, SiLU on ACT,
y ships bf16.  Demod d = rsqrt(Gram-matvec) kept fp32 as before.
"""

import os

import numpy as np

import concourse.bass as bass
import concourse.bacc as bacc
import concourse.mybir as mybir
import concourse.tile as tile
from contextlib import ExitStack
from concourse.bass_utils import run_bass_kernel_spmd

try:
    import ml_dtypes
except ImportError:  # pragma: no cover
    ml_dtypes = None

# ---- problem constants (hardcoded; kernel.py must be self-contained) ----
B, C_IN, C_OUT, H, W, K, NK = 16, 256, 256, 64, 64, 3, 2
N_CORES = 8
S = B // N_CORES            # samples per core
PB = 128                    # partitions per block
IB = C_IN // PB             # input channel blocks (= DoubleRow k-tiles)
OB = C_OUT // PB            # output channel blocks
HW = H * W                  # 4096
PADH, PADW = H + 2, W + 2   # 66, 66
PT = 512                    # pixels per tile (one PSUM bank of fp32)
ROWS_PT = PT // W           # 8 rows per pixel tile
NPT = HW // PT              # 8 pixel tiles
KK = K * K                  # 9
NVEC = 2 * IB + 3 * IB  # a0m/a1m per ib, m2k per ib
EPS = 1e-8
S_W = 128.0                 # PSUM scale for the fp8 hi/lo split
HH = HW // 2                # half-sample pixels (norm phase granularity)

F32 = mybir.dt.float32
F16 = mybir.dt.float16
BF16 = mybir.dt.bfloat16
F8 = mybir.dt.float8e4
I32 = mybir.dt.int32

AF = mybir.ActivationFunctionType
ALU = mybir.AluOpType
PM = mybir.MatmulPerfMode
MAGIC = 0x5F3759DF
# CoreSim does not implement Silu; decompose for sim-only runs
SIM_SILU = os.environ.get("KERNEL_SIM_SILU", "0") == "1"
# CoreSim's DoubleRow exec path rejects 4-dim moving APs; emit equivalent
# per-ib fp8 matmuls for sim-only functional checks
SIM_NO_DR = os.environ.get("KERNEL_SIM_NO_DR", "0") == "1"


def _rsqrt_dve(nc, pool, src_ap, clamp, shape, tag, iters=2):
    """rsqrt(max(src, clamp)) on DVE: bit-trick seed + Newton (fp32)."""
    x = pool.tile(shape, F32, tag=f"{tag}_x", name=f"{tag}_x")
    nc.vector.tensor_scalar_max(out=x, in0=src_ap, scalar1=float(clamp))
    seed = pool.tile(shape, I32, tag=f"{tag}_s", name=f"{tag}_s")
    nc.vector.tensor_scalar(
        out=seed, in0=x.bitcast(I32), scalar1=1, scalar2=None,
        op0=ALU.logical_shift_right,
    )
    nc.vector.tensor_scalar(
        out=seed, in0=seed, scalar1=-1, scalar2=MAGIC,
        op0=ALU.mult, op1=ALU.add,
    )
    r = seed.bitcast(F32)
    xh = pool.tile(shape, F32, tag=f"{tag}_xh", name=f"{tag}_xh")
    nc.vector.tensor_scalar_mul(out=xh, in0=x, scalar1=0.5)
    for it in range(iters):
        t = pool.tile(shape, F32, tag=f"{tag}_t{it}", name=f"{tag}_t{it}")
        nc.vector.tensor_mul(out=t, in0=r, in1=r)
        nc.vector.tensor_mul(out=t, in0=t, in1=xh)
        nc.vector.tensor_scalar(
            out=t, in0=t, scalar1=-1.0, scalar2=1.5, op0=ALU.mult, op1=ALU.add
        )
        r2 = pool.tile(shape, F32, tag=f"{tag}_r{it}", name=f"{tag}_r{it}")
        nc.vector.tensor_mul(out=r2, in0=r, in1=t)
        r = r2
    return r


def build_program():
    nc = bacc.Bacc(trn_type="TRN2", debug=False)

    # x pack: [sample, variant(x8, xlo8), partition, ib*padh*padw] fp8
    x_d = nc.declare_dram_parameter("x8", [S, 2, PB, IB * PADH * PADW], F8,
                                    isOutput=False)
    # weight banks: [bank, ib, partition, kk*C_OUT] bf16 (kk-major inner)
    wt_d = nc.declare_dram_parameter("wT", [NK, IB, PB, KK * C_OUT], BF16,
                                     isOutput=False)
    vecs_d = nc.declare_dram_parameter("vecs", [S, PB, NVEC], F32, isOutput=False)
    smat_d = nc.declare_dram_parameter("smat", [3, IB, PB, C_OUT], F32,
                                       isOutput=False)
    g16_d = nc.declare_dram_parameter("g16", [OB, PB, 1], F32, isOutput=False)
    y_d = nc.declare_dram_parameter("y", [S, PB, OB, HW], BF16, isOutput=True)

    with ExitStack() as ctx:
        tc = ctx.enter_context(tile.TileContext(nc))
        const = ctx.enter_context(tc.tile_pool(name="const", bufs=1))
        small = ctx.enter_context(tc.tile_pool(name="small", bufs=4))
        wch = ctx.enter_context(tc.tile_pool(name="wch", bufs=1))
        wq = ctx.enter_context(tc.tile_pool(name="wq", bufs=2))
        xq = ctx.enter_context(tc.tile_pool(name="xq", bufs=2))
        ycp = ctx.enter_context(tc.tile_pool(name="ycp", bufs=1))
        tp = ctx.enter_context(tc.tile_pool(name="tp", bufs=2))
        np_ = ctx.enter_context(tc.tile_pool(name="normp", bufs=2))
        outp = ctx.enter_context(tc.tile_pool(name="outp", bufs=2))
        pconv = ctx.enter_context(tc.tile_pool(name="pconv", bufs=5, space="PSUM"))
        pdsq = ctx.enter_context(tc.tile_pool(name="pdsq", bufs=2, space="PSUM"))

        # ---- resident constants ----
        wbank = [
            [const.tile([PB, KK * C_OUT], BF16, tag=f"wb{n}{ib}", name=f"wb{n}{ib}")
             for ib in range(IB)]
            for n in range(NK)
        ]
        g16sb = [const.tile([PB, 1], F32, tag=f"g16_{ob}", name=f"g16_{ob}")
                 for ob in range(OB)]
        smat = [
            [const.tile([PB, C_OUT], F32, tag=f"sm{k}{ib}", name=f"sm{k}{ib}")
             for ib in range(IB)]
            for k in range(3)
        ]
        # scalar constants for ACT scale/bias (must be APs)
        c_sw = const.tile([PB, 1], F32, tag="c_sw", name="c_sw")
        nc.vector.memset(c_sw, S_W)
        c_sw16 = const.tile([PB, 1], F32, tag="c_sw16", name="c_sw16")
        nc.vector.memset(c_sw16, S_W / 16.0)
        c_nrm = const.tile([PB, 1], F32, tag="c_nrm", name="c_nrm")
        nc.vector.memset(c_nrm, 1e-6)
        # PE p-state warm-up: ~6.5us of dummy matmuls during the startup
        # DMA window so the first conv tile runs at the full 2.4GHz clock
        dmy = const.tile([PB, 64], F32, tag="dmy", name="dmy")
        nc.vector.memset(dmy, 0.0)
        pwarm = ctx.enter_context(tc.tile_pool(name="pwarm", bufs=1,
                                               space="PSUM"))
        wps = pwarm.tile([1, 64], F32, tag="wps", name="wps")
        for _ in range(40):
            nc.tensor.matmul(wps, lhsT=c_nrm, rhs=dmy, start=True, stop=True)

        # per-sample state produced by the prep phase, consumed by conv/norm
        state = {}

        def prep_front(s):
            """DMAs + weight chain for sample s (no PE work)."""
            vec = small.tile([PB, NVEC], F32, tag="vec", name="vec")
            nc.sync.dma_start(out=vec, in_=vecs_d[s])
            a0m = [vec[:, ib:ib + 1] for ib in range(IB)]
            a1m = [vec[:, IB + ib:IB + ib + 1] for ib in range(IB)]
            m2k = [[vec[:, 2 * IB + 3 * ib + k:2 * IB + 3 * ib + k + 1]
                    for k in range(3)] for ib in range(IB)]

            HOB = KK * PB  # 1152 inner columns per ob half
            if s == 0:
                for ib in range(IB):
                    for n in range(NK):
                        nc.sync.dma_start(out=wbank[n][ib][:, 0:HOB],
                                          in_=wt_d[n, ib, :, 0:HOB])
            # x pack for this sample: [128, v, ib, padh, padw]
            xp = xq.tile([PB, 2, IB, PADH, PADW], F8, tag="xp", name="xp")
            for v in range(2):
                xsrc = x_d[s, v].rearrange("p (i h w) -> p i h w", i=IB, w=PADW)
                if s == 0 and v == 0:
                    # rows 0..17 cover pixel tiles 0-1: start conv sooner
                    nc.sync.dma_start(out=xp[:, v, :, 0:18], in_=xsrc[:, :, 0:18])
                    nc.sync.dma_start(out=xp[:, v, :, 18:PADH], in_=xsrc[:, :, 18:PADH])
                else:
                    nc.sync.dma_start(out=xp[:, v], in_=xsrc)
                if s == 0 and v == 1:
                    for ib in range(IB):
                        for n in range(NK):
                            nc.sync.dma_start(out=wbank[n][ib][:, HOB:2 * HOB],
                                              in_=wt_d[n, ib, :, HOB:2 * HOB])
            if s == 0:
                for k in range(3):
                    for ib in range(IB):
                        nc.sync.dma_start(out=smat[k][ib], in_=smat_d[k, ib])
                for ob in range(OB):
                    nc.sync.dma_start(out=g16sb[ob], in_=g16_d[ob])

            # ---- weight chain: bf16 mix -> fp8 {w8, wlo16, w8_16} ----
            # w8 first (gates the conv start), residuals after.  On the
            # first sample, run per ob-half so conv starts off half the DMA.
            w8 = wq.tile([PB, IB, KK * C_OUT], F8, tag="w8", name="w8")
            wlo = wq.tile([PB, IB, KK * C_OUT], F8, tag="wlo", name="wlo")
            w816 = wq.tile([PB, IB, KK * C_OUT], F8, tag="w816", name="w816")
            cols = [slice(ob * HOB, (ob + 1) * HOB) for ob in range(OB)] \
                if s == 0 else [slice(0, OB * HOB)]
            wms = {}
            for cs in cols:
                for ib in range(IB):
                    m1 = wch.tile([PB, KK * C_OUT], BF16, tag="wa", name=f"m1_{ib}")
                    nc.vector.tensor_scalar_mul(out=m1[:, cs],
                                                in0=wbank[0][ib][:, cs],
                                                scalar1=a0m[ib])
                    m2 = wch.tile([PB, KK * C_OUT], BF16, tag="wb", name=f"m2_{ib}")
                    nc.vector.tensor_scalar_mul(out=m2[:, cs],
                                                in0=wbank[1][ib][:, cs],
                                                scalar1=a1m[ib])
                    wm = wch.tile([PB, KK * C_OUT], BF16, tag=f"wc{ib}",
                                  name=f"wm_{ib}")
                    nc.vector.tensor_add(out=wm[:, cs], in0=m1[:, cs],
                                         in1=m2[:, cs])
                    if s == 0 and cs.start == 0 and ib == 1:
                        # DVE quant overlaps ACT's ib0 quant: earlier conv start
                        nc.vector.tensor_scalar_mul(out=w8[:, ib, cs],
                                                    in0=wm[:, cs], scalar1=S_W)
                    else:
                        nc.scalar.activation(out=w8[:, ib, cs], in_=wm[:, cs],
                                             func=AF.Copy, scale=c_sw)
                    wms[ib] = wm
                for ib in range(IB):
                    wm = wms[ib]
                    nc.scalar.activation(out=w816[:, ib, cs], in_=wm[:, cs],
                                         func=AF.Copy, scale=c_sw16)
                    w8n = wch.tile([PB, KK * C_OUT], BF16, tag="wa",
                                   name=f"w8n_{ib}")
                    nc.vector.tensor_scalar_mul(out=w8n[:, cs], in0=w8[:, ib, cs],
                                                scalar1=-1.0 / S_W)
                    rres = wch.tile([PB, KK * C_OUT], BF16, tag="wb",
                                    name=f"rr_{ib}")
                    nc.vector.tensor_add(out=rres[:, cs], in0=wm[:, cs],
                                         in1=w8n[:, cs])
                    nc.vector.tensor_scalar_mul(out=wlo[:, ib, cs],
                                                in0=rres[:, cs], scalar1=S_W)
            state[s] = dict(xp=xp, w8=w8, wlo=wlo, w816=w816, m2k=m2k)

        def prep_demod(s):
            """dsq Gram matvecs (PE) + d' rsqrt (DVE) for sample s."""
            m2k = state[s]["m2k"]
            dsq_ps = pdsq.tile([PB, OB], F32, tag="dsq", name="dsq")
            for ob in range(OB):
                i_mv = 0
                for ib in range(IB):
                    for k in range(3):
                        nc.tensor.matmul(
                            dsq_ps[:, ob:ob + 1],
                            lhsT=smat[k][ib][:, ob * PB:(ob + 1) * PB],
                            rhs=m2k[ib][k],
                            start=(i_mv == 0),
                            stop=(i_mv == 3 * IB - 1),
                        )
                        i_mv += 1
            dcol = []
            for ob in range(OB):
                d = _rsqrt_dve(nc, small, dsq_ps[:, ob:ob + 1],
                               EPS * S_W * S_W, [PB, 1], f"d{ob}")
                dcol.append(d)
            state[s]["dcol"] = dcol

        def emit_group(s, pt, ob, ps, terms, start, stop, half=None):
            st = state[s]
            xp = st["xp"]
            r0, nr = (0, ROWS_PT) if half is None else (half[1], half[2])
            n_mm = len(terms) * KK * (IB if SIM_NO_DR else 1)
            i_mm = 0
            for wt_name, v in terms:
                wt = st[wt_name]
                for ki in range(K):
                    for kj in range(K):
                        kk = ki * K + kj
                        pso = ps if half is None else ps[:, 0:nr * W]
                        if SIM_NO_DR:
                            for ib in range(IB):
                                nc.tensor.matmul(
                                    pso,
                                    lhsT=wt[:, ib, (ob * KK + kk) * PB:
                                            (ob * KK + kk) * PB + PB],
                                    rhs=xp[:, v, ib,
                                           pt * ROWS_PT + r0 + ki:
                                           pt * ROWS_PT + r0 + ki + nr,
                                           kj: kj + W],
                                    start=(start and i_mm == 0),
                                    stop=(stop and i_mm == n_mm - 1),
                                )
                                i_mm += 1
                            continue
                        lhsT = wt[:, :, (ob * KK + kk) * PB:
                                  (ob * KK + kk) * PB + PB]
                        rhs = xp[:, v, :,
                                 pt * ROWS_PT + r0 + ki:
                                 pt * ROWS_PT + r0 + ki + nr,
                                 kj: kj + W]
                        nc.tensor.matmul(
                            pso, lhsT=lhsT, rhs=rhs,
                            start=(start and i_mm == 0),
                            stop=(stop and i_mm == n_mm - 1),
                            perf_mode=PM.DoubleRow,
                        )
                        i_mm += 1

        T_MAIN = [("w8", 0), ("wlo", 0)]
        T_X = [("w816", 1)]

        def conv_tile(s, pt, ob, yc, drain=True, half=None):
            ps = pconv.tile([PB, PT], F32, tag="conv", name="conv") \
                if half is None else half[0]
            emit_group(s, pt, ob, ps, T_MAIN + T_X, True, True, half=half)
            if drain:
                # fold the demod scale into the drain for s>0 (sample 0's
                # dcol is not ready when its early tiles drain; its phases
                # apply the scale instead)
                sc = state[s]["dcol"][ob] if s > 0 else 1.0
                nc.scalar.activation(out=yc[ob][:, pt * PT:(pt + 1) * PT],
                                     in_=ps, func=AF.Copy, scale=sc)
            return ps

        def conv_tiles(s, pt_range, yc, obs=(0, 1)):
            for pt in pt_range:
                for ob in obs:
                    conv_tile(s, pt, ob, yc)

        def conv_tiles_deferred_x(s, pts, ob, yc):
            """T1+T2 of each tile first, T3 after: hides the xlo DMA."""
            pss = {}
            for pt in pts:
                pss[pt] = pconv.tile([PB, PT], F32, tag="conv", name="conv")
                emit_group(s, pt, ob, pss[pt], T_MAIN, True, False)
            for pt in pts:
                emit_group(s, pt, ob, pss[pt], T_X, False, True)
                nc.scalar.activation(out=yc[ob][:, pt * PT:(pt + 1) * PT],
                                     in_=pss[pt], func=AF.Copy)

        def norm_phase(s, p0, p1, yc, ps_direct=None, scaled_yc=False,
                       final=False):
            """RMS-norm + SiLU + store for pixel range [p0, p1).

            Engine split: q-squares on ACT and ns/z on Pool for steady
            phases (DVE is the scarce engine); the final phase minimizes
            chain latency instead. rsqrt is a DVE f16 bit-trick + Newton,
            so ACT never reloads its function table.
            """
            st = state[s]
            hs = slice(p0, p1)
            n = p1 - p0
            tt, tg = [], []
            for ob in range(OB):
                if ps_direct is not None:
                    t = tp.tile([PB, n], F16, tag=f"t{ob}", name=f"t{ob}")
                    nc.vector.tensor_scalar_mul(out=t, in0=ps_direct[ob],
                                                scalar1=st["dcol"][ob])
                elif scaled_yc:
                    t = yc[ob][:, hs]
                else:
                    t = tp.tile([PB, n], F16, tag=f"t{ob}", name=f"t{ob}")
                    nc.vector.tensor_scalar_mul(out=t, in0=yc[ob][:, hs],
                                                scalar1=st["dcol"][ob])
                tt.append(t)
            q0 = np_.tile([PB, n], F16, tag="q", name="q0")
            q1 = np_.tile([PB, n], F16, tag="q", name="q1")
            if final:
                nc.vector.tensor_mul(out=q0, in0=tt[0], in1=tt[0])
                nc.vector.tensor_mul(out=q1, in0=tt[1], in1=tt[1])
            else:
                nc.scalar.activation(out=q0, in_=tt[0], func=AF.Square)
                nc.scalar.activation(out=q1, in_=tt[1], func=AF.Square)
            nsum = np_.tile([PB, n], F16, tag="nsum", name="nsum")
            if final:
                nc.vector.tensor_add(out=nsum, in0=q0, in1=q1)
            else:
                nc.gpsimd.tensor_add(out=nsum, in0=q0, in1=q1)
            # t*g16 off the critical path (before the partition reduce)
            for ob in range(OB):
                g = tp.tile([PB, n], F16, tag=f"tg{ob}", name=f"tg{ob}")
                nc.vector.tensor_scalar_mul(out=g, in0=tt[ob],
                                            scalar1=g16sb[ob])
                tg.append(g)
            nc.gpsimd.partition_all_reduce(
                nsum[:], nsum[:], PB, bass.bass_isa.ReduceOp.add
            )
            # f16 bit-trick rsqrt + 1 Newton iter, all on DVE
            I16 = mybir.dt.int16
            rt = np_.tile([PB, n], F16, tag="rt", name="rt")
            sd = rt.bitcast(I16)
            nc.vector.tensor_scalar(
                out=sd, in0=nsum.bitcast(I16), scalar1=1, scalar2=None,
                op0=ALU.logical_shift_right,
            )
            nc.vector.tensor_scalar(
                out=sd, in0=sd, scalar1=-1, scalar2=0x59BA,
                op0=ALU.mult, op1=ALU.add,
            )
            tn = np_.tile([PB, n], F16, tag="tn", name="tn")
            nc.vector.tensor_mul(out=tn, in0=rt, in1=rt)
            nc.vector.tensor_mul(out=tn, in0=tn, in1=nsum)
            nc.vector.tensor_scalar(
                out=tn, in0=tn, scalar1=-0.5, scalar2=1.5,
                op0=ALU.mult, op1=ALU.add,
            )
            nc.vector.tensor_mul(out=rt, in0=rt, in1=tn)
            o = outp.tile([PB, OB, n], BF16, tag="o", name="o")
            for ob in range(OB):
                # z overwrites tg in place (tg dead after this)
                if final and ob == 1:
                    nc.vector.tensor_mul(out=tg[ob], in0=tg[ob], in1=rt)
                else:
                    nc.gpsimd.tensor_mul(out=tg[ob], in0=tg[ob], in1=rt)
                if SIM_SILU:
                    sg = np_.tile([PB, n], F16, tag="tn", name=f"sg{ob}")
                    nc.scalar.activation(out=sg, in_=tg[ob], func=AF.Sigmoid)
                    nc.vector.tensor_mul(out=o[:, ob], in0=tg[ob], in1=sg)
                else:
                    nc.scalar.activation(out=o[:, ob], in_=tg[ob], func=AF.Silu)
            # one fused DMA for both ob halves (fewer HWDGE holds); final
            # phases issue from less-contended queues
            if final:
                nc.scalar.dma_start(out=y_d[s, :, :, hs], in_=o)
            else:
                nc.sync.dma_start(out=y_d[s, :, :, hs], in_=o)

        # ---- main schedule ----
        QQ = HW // 4
        prep_front(0)
        yc0 = [ycp.tile([PB, HW], F16, tag=f"yc{ob}", name=f"yc{ob}")
               for ob in range(OB)]
        # s0: ob0 first (its weights land first), T3 deferred on the first
        # two tiles to ride out the xlo DMA.
        conv_tiles_deferred_x(0, [0, 1], 0, yc0)
        conv_tiles(0, range(2, NPT), yc0, obs=(0,))
        prep_demod(0)
        conv_tiles(0, range(0, 4), yc0, obs=(1,))
        if S > 1:
            prep_front(1)
            prep_demod(1)
        norm_phase(0, 0, HH, yc0)
        conv_tiles(0, range(4, NPT), yc0, obs=(1,))
        norm_phase(0, HH, HW, yc0)
        for s in range(1, S):
            yc = [ycp.tile([PB, HW], F16, tag=f"yc{ob}", name=f"yc{ob}")
                  for ob in range(OB)]
            if s + 1 < S:
                conv_tiles(s, range(0, 4), yc)
                prep_front(s + 1)
                prep_demod(s + 1)
                norm_phase(s, 0, HH, yc, scaled_yc=True)
                conv_tiles(s, range(4, NPT), yc)
                norm_phase(s, HH, HW, yc, scaled_yc=True)
            else:
                # last sample: spread phases so only the small final one
                # trails the conv
                conv_tiles(s, range(0, 2), yc)
                conv_tiles(s, range(2, 4), yc)
                norm_phase(s, 0, QQ, yc, scaled_yc=True)
                conv_tiles(s, range(4, 6), yc)
                norm_phase(s, QQ, HH, yc, scaled_yc=True)
                conv_tiles(s, range(6, 7), yc)
                norm_phase(s, HH, 3 * QQ, yc, scaled_yc=True)
                ps7a = [pconv.tile([PB, PT // 2], F32, tag="conv", name="c7a")
                        for _ in range(OB)]
                for ob in range(OB):
                    conv_tile(s, 7, ob, yc, drain=False, half=(ps7a[ob], 0, 4))
                ps7b = [pconv.tile([PB, PT // 2], F32, tag="conv", name="c7b")
                        for _ in range(OB)]
                for ob in range(OB):
                    conv_tile(s, 7, ob, yc, drain=False, half=(ps7b[ob], 4, 4))
                norm_phase(s, 3 * QQ, 7 * HW // 8, yc, scaled_yc=True)
                norm_phase(s, 7 * HW // 8, 15 * HW // 16, yc, ps_direct=ps7a,
                           final=True)
                norm_phase(s, 15 * HW // 16, HW, yc, ps_direct=ps7b,
                           final=True)
    nc.finalize()
    return nc


_NC_CACHE = {}


def _get_program():
    if "nc" not in _NC_CACHE:
        _NC_CACHE["nc"] = build_program()
    return _NC_CACHE["nc"]


def _host_prep(x, mod, kernel_mod, weights, gamma):
    assert ml_dtypes is not None, "ml_dtypes required for fp8 host prep"
    f8 = ml_dtypes.float8_e4m3
    bf = ml_dtypes.bfloat16

    x = np.asarray(x, dtype=np.float32)
    mod = np.asarray(mod, dtype=np.float32)
    kernel_mod = np.asarray(kernel_mod, dtype=np.float32)
    weights = np.asarray(weights, dtype=np.float32)
    gamma = np.asarray(gamma, dtype=np.float32)

    e = np.exp(kernel_mod - kernel_mod.max(axis=-1, keepdims=True))
    attn = (e / e.sum(axis=-1, keepdims=True)).astype(np.float32)     # [B, NK]
    modp1 = mod + 1.0

    # weights -> [NK, I, OB, KK, 128] ob-major inner, bf16
    # (transpose axes: [n, o, i, kh, kw] -> [n, i, ob, kh*kw, o128])
    w6 = weights.reshape(NK, OB, PB, C_IN, K, K)
    wT = np.ascontiguousarray(
        w6.transpose(0, 3, 1, 4, 5, 2).reshape(NK, IB, PB, OB * KK * PB)
    ).astype(bf)

    # bank Gram stats over kk: [i, o], scaled by S_W^2
    wio = weights.transpose(0, 2, 1, 3, 4).reshape(NK, C_IN, C_OUT, KK)
    s00 = (wio[0] * wio[0]).sum(-1)
    s01 = (wio[0] * wio[1]).sum(-1)
    s11 = (wio[1] * wio[1]).sum(-1)
    smat = np.ascontiguousarray(
        (np.stack([s00, s01, s11]) * (S_W * S_W))
        .reshape(3, IB, PB, C_OUT).astype(np.float32)
    )
    g16 = np.ascontiguousarray(
        (gamma * np.sqrt(C_OUT)).astype(np.float32).reshape(OB, PB, 1)
    )

    # x variants: fp8 hi + fp8 residual(x16), zero-padded, per-partition pack
    x8 = x.astype(f8)
    xlo = ((x - x8.astype(np.float32)) * 16.0).astype(f8)
    xpack = np.zeros((B, 2, IB, PB, PADH, PADW), dtype=f8)
    xpack[:, 0, :, :, 1:H + 1, 1:W + 1] = x8.reshape(B, IB, PB, H, W)
    xpack[:, 1, :, :, 1:H + 1, 1:W + 1] = xlo.reshape(B, IB, PB, H, W)
    # -> [B, v, PB, ib*padh*padw] (partition-major for a single DMA per v)
    xpack = np.ascontiguousarray(
        xpack.transpose(0, 1, 3, 2, 4, 5).reshape(B, 2, PB, IB * PADH * PADW)
    )

    in_maps = []
    for c in range(N_CORES):
        vecs = np.empty((S, PB, NVEC), np.float32)
        for si in range(S):
            b = c * S + si
            a0, a1 = attn[b, 0], attn[b, 1]
            mp = modp1[b].reshape(IB, PB)
            m2b = (modp1[b] * modp1[b]).reshape(IB, PB)
            for ib in range(IB):
                vecs[si, :, ib] = a0 * mp[ib]
                vecs[si, :, IB + ib] = a1 * mp[ib]
                vecs[si, :, 2 * IB + 3 * ib + 0] = m2b[ib] * (a0 * a0)
                vecs[si, :, 2 * IB + 3 * ib + 1] = m2b[ib] * (2.0 * a0 * a1)
                vecs[si, :, 2 * IB + 3 * ib + 2] = m2b[ib] * (a1 * a1)
        sl = slice(c * S, (c + 1) * S)
        in_maps.append({
            "x8": xpack[sl],
            "wT": wT,
            "smat": smat,
            "vecs": vecs,
            "g16": g16,
        })
    return in_maps


def kernel(x, mod, kernel_mod, weights, gamma, _trace=False, _trace_kwargs=None):
    nc = _get_program()
    in_maps = _host_prep(x, mod, kernel_mod, weights, gamma)
    res = run_bass_kernel_spmd(
        nc, in_maps, list(range(N_CORES)),
        trace=_trace, **(_trace_kwargs or {}),
    )
    y = np.concatenate(
        [np.asarray(res.results[c]["y"]).astype(np.float32)
         .reshape(S, PB, OB, HW).transpose(0, 2, 1, 3)
         .reshape(S, C_OUT, H, W) for c in range(N_CORES)],
        axis=0,
    )
    if _trace:
        kernel.last_results = res
    return y


kernel.last_results = None


# revision 38
# speedup vs baseline: 1.5493x; 1.0075x over previous
"""Trainium2 Bass kernel for nn_Block_40742059770386 (dense_cnn).

Per-sample adaptively-mixed, style-modulated, demodulated 3x3 conv
(StyleGAN2-style) + channel RMS norm + SiLU.

Sharding: data-parallel over batch. B=16 samples -> 8 cores x 2 samples.
The kernel bank (2 x 256 x 256 x 3 x 3, shipped bf16) and gamma are
replicated.

Conv engine strategy (the PE-time floor):
  fp8e4m3 DoubleRow matmuls contract all 256 input channels per
  instruction at 0.5 cycles/row -- 4x the f32r row rate.  Raw fp8 on
  both operands misses the 2e-2 tolerance (~4e-2), so the conv runs a
  3-term hi/lo split at a shared PSUM scale S=128:

      S*w*x ~= w8@x8 + wlo16@x8 + w8_16@xlo8
      w8    = fp8(S*wm)            (wm = bf16-mixed modulated weights)
      wlo16 = fp8(S*wm - w8)       (weight residual, same S scale)
      w8_16 = fp8(S/16*wm)         (for the x-residual term)
      x8    = fp8(x), xlo8 = fp8(16*(x - x8))   (host-side dtype prep)

  27 DoubleRow matmuls per [128 out, 512 px] tile vs 18 f32r ones:
  1.33x less PE time at ~bf16 accuracy (measured 4.6e-3 end to end).

Modulation is folded into the mix coefficients (a_n * (mod+1)[ci] as
per-partition scalar columns), so x ships unmodulated and pre-padded;
no on-chip modulate/pad pass at all.

Epilogue: Pool drains PSUM -> f16 (427ns/tile), everything else in f16
on DVE at 2x/4x rates; channel-sum via gpsimd partition_all_reduce
(fp32 internal), rsqrt = ACT Sqrt + DVE reciprocal, SiLU on ACT,
y ships bf16.  Demod d = rsqrt(Gram-matvec) kept fp32 as before.
"""

import os

import numpy as np

import concourse.bass as bass
import concourse.bacc as bacc
import concourse.mybir as mybir
import concourse.tile as tile
from contextlib import ExitStack
from concourse.bass_utils import run_bass_kernel_spmd

try:
    import ml_dtypes
except ImportError:  # pragma: no cover
    ml_dtypes = None

# ---- problem constants (hardcoded; kernel.py must be self-contained) ----
B, C_IN, C_OUT, H, W, K, NK = 16, 256, 256, 64, 64, 3, 2
N_CORES = 8
S = B // N_CORES            # samples per core
PB = 128                    # partitions per block
IB = C_IN // PB             # input channel blocks (= DoubleRow k-tiles)
OB = C_OUT // PB            # output channel blocks
HW = H * W                  # 4096
PADH, PADW = H + 2, W + 2   # 66, 66
PT = 512                    # pixels per tile (one PSUM bank of fp32)
ROWS_PT = PT // W           # 8 rows per pixel tile
NPT = HW // PT              # 8 pixel tiles
KK = K * K                  # 9
NVEC = 2 * IB + 3 * IB  # a0m/a1m per ib, m2k per ib
EPS = 1e-8
S_W = 128.0                 # PSUM scale for the fp8 hi/lo split
HH = HW // 2                # half-sample pixels (norm phase granularity)

F32 = mybir.dt.float32
F16 = mybir.dt.float16
BF16 = mybir.dt.bfloat16
F8 = mybir.dt.float8e4
I32 = mybir.dt.int32

AF = mybir.ActivationFunctionType
ALU = mybir.AluOpType
PM = mybir.MatmulPerfMode
MAGIC = 0x5F3759DF
# CoreSim does not implement Silu; decompose for sim-only runs
SIM_SILU = os.environ.get("KERNEL_SIM_SILU", "0") == "1"
# CoreSim's DoubleRow exec path rejects 4-dim moving APs; emit equivalent
# per-ib fp8 matmuls for sim-only functional checks
SIM_NO_DR = os.environ.get("KERNEL_SIM_NO_DR", "0") == "1"


def _rsqrt_dve(nc, pool, src_ap, clamp, shape, tag, iters=2):
    """rsqrt(max(src, clamp)) on DVE: bit-trick seed + Newton (fp32)."""
    x = pool.tile(shape, F32, tag=f"{tag}_x", name=f"{tag}_x")
    nc.vector.tensor_scalar_max(out=x, in0=src_ap, scalar1=float(clamp))
    seed = pool.tile(shape, I32, tag=f"{tag}_s", name=f"{tag}_s")
    nc.vector.tensor_scalar(
        out=seed, in0=x.bitcast(I32), scalar1=1, scalar2=None,
        op0=ALU.logical_shift_right,
    )
    nc.vector.tensor_scalar(
        out=seed, in0=seed, scalar1=-1, scalar2=MAGIC,
        op0=ALU.mult, op1=ALU.add,
    )
    r = seed.bitcast(F32)
    xh = pool.tile(shape, F32, tag=f"{tag}_xh", name=f"{tag}_xh")
    nc.vector.tensor_scalar_mul(out=xh, in0=x, scalar1=0.5)
    for it in range(iters):
        t = pool.tile(shape, F32, tag=f"{tag}_t{it}", name=f"{tag}_t{it}")
        nc.vector.tensor_mul(out=t, in0=r, in1=r)
        nc.vector.tensor_mul(out=t, in0=t, in1=xh)
        nc.vector.tensor_scalar(
            out=t, in0=t, scalar1=-1.0, scalar2=1.5, op0=ALU.mult, op1=ALU.add
        )
        r2 = pool.tile(shape, F32, tag=f"{tag}_r{it}", name=f"{tag}_r{it}")
        nc.vector.tensor_mul(out=r2, in0=r, in1=t)
        r = r2
    return r


def build_program():
    nc = bacc.Bacc(trn_type="TRN2", debug=False)

    # x pack: [sample, variant(x8, xlo8), partition, ib*padh*padw] fp8
    x_d = nc.declare_dram_parameter("x8", [S, 2, PB, IB * PADH * PADW], F8,
                                    isOutput=False)
    # weight banks: [bank, ib, partition, ob*kk*128 (+vecs rider)] bf16
    WCOLS = KK * C_OUT + 2 * NVEC
    wt_d = nc.declare_dram_parameter("wT", [NK, IB, PB, WCOLS], BF16,
                                     isOutput=False)
    vecs_d = nc.declare_dram_parameter("vecs", [S, PB, NVEC], F32, isOutput=False)
    smat_d = nc.declare_dram_parameter("smat", [3, IB, PB, C_OUT], F32,
                                       isOutput=False)
    g16_d = nc.declare_dram_parameter("g16", [OB, PB, 1], F32, isOutput=False)
    y_d = nc.declare_dram_parameter("y", [S, PB, OB, HW], BF16, isOutput=True)

    with ExitStack() as ctx:
        tc = ctx.enter_context(tile.TileContext(nc))
        const = ctx.enter_context(tc.tile_pool(name="const", bufs=1))
        small = ctx.enter_context(tc.tile_pool(name="small", bufs=4))
        wch = ctx.enter_context(tc.tile_pool(name="wch", bufs=1))
        wq = ctx.enter_context(tc.tile_pool(name="wq", bufs=2))
        xq = ctx.enter_context(tc.tile_pool(name="xq", bufs=2))
        ycp = ctx.enter_context(tc.tile_pool(name="ycp", bufs=1))
        tp = ctx.enter_context(tc.tile_pool(name="tp", bufs=2))
        np_ = ctx.enter_context(tc.tile_pool(name="normp", bufs=2))
        outp = ctx.enter_context(tc.tile_pool(name="outp", bufs=2))
        pconv = ctx.enter_context(tc.tile_pool(name="pconv", bufs=5, space="PSUM"))
        pdsq = ctx.enter_context(tc.tile_pool(name="pdsq", bufs=2, space="PSUM"))

        # ---- resident constants ----
        wbank = [
            [const.tile([PB, WCOLS], BF16, tag=f"wb{n}{ib}", name=f"wb{n}{ib}")
             for ib in range(IB)]
            for n in range(NK)
        ]
        g16sb = [const.tile([PB, 1], F32, tag=f"g16_{ob}", name=f"g16_{ob}")
                 for ob in range(OB)]
        smat = [
            [const.tile([PB, C_OUT], F32, tag=f"sm{k}{ib}", name=f"sm{k}{ib}")
             for ib in range(IB)]
            for k in range(3)
        ]
        # scalar constants for ACT scale/bias (must be APs)
        c_sw = const.tile([PB, 1], F32, tag="c_sw", name="c_sw")
        nc.vector.memset(c_sw, S_W)
        c_sw16 = const.tile([PB, 1], F32, tag="c_sw16", name="c_sw16")
        nc.vector.memset(c_sw16, S_W / 16.0)
        c_nrm = const.tile([PB, 1], F32, tag="c_nrm", name="c_nrm")
        nc.vector.memset(c_nrm, 1e-6)
        # PE p-state warm-up: ~6.5us of dummy matmuls during the startup
        # DMA window so the first conv tile runs at the full 2.4GHz clock
        dmy = const.tile([PB, 64], F32, tag="dmy", name="dmy")
        nc.vector.memset(dmy, 0.0)
        pwarm = ctx.enter_context(tc.tile_pool(name="pwarm", bufs=1,
                                               space="PSUM"))
        wps = pwarm.tile([1, 64], F32, tag="wps", name="wps")
        for _ in range(40):
            nc.tensor.matmul(wps, lhsT=c_nrm, rhs=dmy, start=True, stop=True)

        # per-sample state produced by the prep phase, consumed by conv/norm
        state = {}

        def prep_front(s):
            """DMAs + weight chain for sample s (no PE work)."""
            if s == 0:
                # s0's vecs ride in the first weight DMA (bitcast rider
                # columns) -- one less DMA on the startup critical path
                vec = wbank[0][0][:, KK * C_OUT:WCOLS].bitcast(F32)
            else:
                vec = small.tile([PB, NVEC], F32, tag="vec", name="vec")
                nc.sync.dma_start(out=vec, in_=vecs_d[s])
            a0m = [vec[:, ib:ib + 1] for ib in range(IB)]
            a1m = [vec[:, IB + ib:IB + ib + 1] for ib in range(IB)]
            m2k = [[vec[:, 2 * IB + 3 * ib + k:2 * IB + 3 * ib + k + 1]
                    for k in range(3)] for ib in range(IB)]

            HOB = KK * PB  # 1152 inner columns per ob half
            if s == 0:
                nc.sync.dma_start(out=wbank[0][0][:, KK * C_OUT:WCOLS],
                                  in_=wt_d[0, 0, :, KK * C_OUT:WCOLS])
                for ib in range(IB):
                    for n in range(NK):
                        nc.sync.dma_start(out=wbank[n][ib][:, 0:HOB],
                                          in_=wt_d[n, ib, :, 0:HOB])
            # x pack for this sample: [128, v, ib, padh, padw]
            xp = xq.tile([PB, 2, IB, PADH, PADW], F8, tag="xp", name="xp")
            for v in range(2):
                xsrc = x_d[s, v].rearrange("p (i h w) -> p i h w", i=IB, w=PADW)
                if s == 0 and v == 0:
                    # rows 0..17 cover pixel tiles 0-1: start conv sooner
                    nc.sync.dma_start(out=xp[:, v, :, 0:18], in_=xsrc[:, :, 0:18])
                    nc.sync.dma_start(out=xp[:, v, :, 18:PADH], in_=xsrc[:, :, 18:PADH])
                else:
                    nc.sync.dma_start(out=xp[:, v], in_=xsrc)
                if s == 0 and v == 1:
                    for ib in range(IB):
                        for n in range(NK):
                            nc.sync.dma_start(out=wbank[n][ib][:, HOB:2 * HOB],
                                              in_=wt_d[n, ib, :, HOB:2 * HOB])
            if s == 0:
                for k in range(3):
                    for ib in range(IB):
                        nc.sync.dma_start(out=smat[k][ib], in_=smat_d[k, ib])
                for ob in range(OB):
                    nc.sync.dma_start(out=g16sb[ob], in_=g16_d[ob])

            # ---- weight chain: bf16 mix -> fp8 {w8, wlo16, w8_16} ----
            # w8 first (gates the conv start), residuals after.  On the
            # first sample, run per ob-half so conv starts off half the DMA.
            w8 = wq.tile([PB, IB, KK * C_OUT], F8, tag="w8", name="w8")
            wlo = wq.tile([PB, IB, KK * C_OUT], F8, tag="wlo", name="wlo")
            w816 = wq.tile([PB, IB, KK * C_OUT], F8, tag="w816", name="w816")
            cols = [slice(ob * HOB, (ob + 1) * HOB) for ob in range(OB)] \
                if s == 0 else [slice(0, OB * HOB)]
            wms = {}
            for cs in cols:
                for ib in range(IB):
                    m1 = wch.tile([PB, KK * C_OUT], BF16, tag="wa", name=f"m1_{ib}")
                    nc.vector.tensor_scalar_mul(out=m1[:, cs],
                                                in0=wbank[0][ib][:, cs],
                                                scalar1=a0m[ib])
                    m2 = wch.tile([PB, KK * C_OUT], BF16, tag="wb", name=f"m2_{ib}")
                    nc.vector.tensor_scalar_mul(out=m2[:, cs],
                                                in0=wbank[1][ib][:, cs],
                                                scalar1=a1m[ib])
                    wm = wch.tile([PB, KK * C_OUT], BF16, tag=f"wc{ib}",
                                  name=f"wm_{ib}")
                    nc.vector.tensor_add(out=wm[:, cs], in0=m1[:, cs],
                                         in1=m2[:, cs])
                    if s == 0 and cs.start == 0 and ib == 1:
                        # DVE quant overlaps ACT's ib0 quant: earlier conv start
                        nc.vector.tensor_scalar_mul(out=w8[:, ib, cs],
                                                    in0=wm[:, cs], scalar1=S_W)
                    else:
                        nc.scalar.activation(out=w8[:, ib, cs], in_=wm[:, cs],
                                             func=AF.Copy, scale=c_sw)
                    wms[ib] = wm
                for ib in range(IB):
                    wm = wms[ib]
                    nc.scalar.activation(out=w816[:, ib, cs], in_=wm[:, cs],
                                         func=AF.Copy, scale=c_sw16)
                    w8n = wch.tile([PB, KK * C_OUT], BF16, tag="wa",
                                   name=f"w8n_{ib}")
                    nc.vector.tensor_scalar_mul(out=w8n[:, cs], in0=w8[:, ib, cs],
                                                scalar1=-1.0 / S_W)
                    rres = wch.tile([PB, KK * C_OUT], BF16, tag="wb",
                                    name=f"rr_{ib}")
                    nc.vector.tensor_add(out=rres[:, cs], in0=wm[:, cs],
                                         in1=w8n[:, cs])
                    nc.vector.tensor_scalar_mul(out=wlo[:, ib, cs],
                                                in0=rres[:, cs], scalar1=S_W)
            state[s] = dict(xp=xp, w8=w8, wlo=wlo, w816=w816, m2k=m2k)

        def prep_demod(s):
            """dsq Gram matvecs (PE) + d' rsqrt (DVE) for sample s."""
            m2k = state[s]["m2k"]
            dsq_ps = pdsq.tile([PB, OB], F32, tag="dsq", name="dsq")
            for ob in range(OB):
                i_mv = 0
                for ib in range(IB):
                    for k in range(3):
                        nc.tensor.matmul(
                            dsq_ps[:, ob:ob + 1],
                            lhsT=smat[k][ib][:, ob * PB:(ob + 1) * PB],
                            rhs=m2k[ib][k],
                            start=(i_mv == 0),
                            stop=(i_mv == 3 * IB - 1),
                        )
                        i_mv += 1
            dcol = []
            for ob in range(OB):
                d = _rsqrt_dve(nc, small, dsq_ps[:, ob:ob + 1],
                               EPS * S_W * S_W, [PB, 1], f"d{ob}")
                dcol.append(d)
            state[s]["dcol"] = dcol

        def emit_group(s, pt, ob, ps, terms, start, stop, half=None):
            st = state[s]
            xp = st["xp"]
            r0, nr = (0, ROWS_PT) if half is None else (half[1], half[2])
            n_mm = len(terms) * KK * (IB if SIM_NO_DR else 1)
            i_mm = 0
            for wt_name, v in terms:
                wt = st[wt_name]
                for ki in range(K):
                    for kj in range(K):
                        kk = ki * K + kj
                        pso = ps if half is None else ps[:, 0:nr * W]
                        if SIM_NO_DR:
                            for ib in range(IB):
                                nc.tensor.matmul(
                                    pso,
                                    lhsT=wt[:, ib, (ob * KK + kk) * PB:
                                            (ob * KK + kk) * PB + PB],
                                    rhs=xp[:, v, ib,
                                           pt * ROWS_PT + r0 + ki:
                                           pt * ROWS_PT + r0 + ki + nr,
                                           kj: kj + W],
                                    start=(start and i_mm == 0),
                                    stop=(stop and i_mm == n_mm - 1),
                                )
                                i_mm += 1
                            continue
                        lhsT = wt[:, :, (ob * KK + kk) * PB:
                                  (ob * KK + kk) * PB + PB]
                        rhs = xp[:, v, :,
                                 pt * ROWS_PT + r0 + ki:
                                 pt * ROWS_PT + r0 + ki + nr,
                                 kj: kj + W]
                        nc.tensor.matmul(
                            pso, lhsT=lhsT, rhs=rhs,
                            start=(start and i_mm == 0),
                            stop=(stop and i_mm == n_mm - 1),
                            perf_mode=PM.DoubleRow,
                        )
                        i_mm += 1

        T_MAIN = [("w8", 0), ("wlo", 0)]
        T_X = [("w816", 1)]

        def conv_tile(s, pt, ob, yc, drain=True, half=None):
            ps = pconv.tile([PB, PT], F32, tag="conv", name="conv") \
                if half is None else half[0]
            emit_group(s, pt, ob, ps, T_MAIN + T_X, True, True, half=half)
            if drain:
                # fold the demod scale into the drain for s>0 (sample 0's
                # dcol is not ready when its early tiles drain; its phases
                # apply the scale instead)
                sc = state[s]["dcol"][ob] if s > 0 else 1.0
                p0 = pt * PT if half is None else pt * PT + half[1] * W
                n = PT if half is None else half[2] * W
                nc.scalar.activation(out=yc[ob][:, p0:p0 + n],
                                     in_=ps[:, 0:n], func=AF.Copy, scale=sc)
            return ps

        def conv_tiles(s, pt_range, yc, obs=(0, 1)):
            for pt in pt_range:
                for ob in obs:
                    conv_tile(s, pt, ob, yc)

        def conv_tiles_deferred_x(s, pts, ob, yc):
            """T1+T2 of each tile first, T3 after: hides the xlo DMA."""
            pss = {}
            for pt in pts:
                pss[pt] = pconv.tile([PB, PT], F32, tag="conv", name="conv")
                emit_group(s, pt, ob, pss[pt], T_MAIN, True, False)
            for pt in pts:
                emit_group(s, pt, ob, pss[pt], T_X, False, True)
                nc.scalar.activation(out=yc[ob][:, pt * PT:(pt + 1) * PT],
                                     in_=pss[pt], func=AF.Copy)

        def norm_phase(s, p0, p1, yc, ps_direct=None, scaled_yc=False,
                       final=False):
            """RMS-norm + SiLU + store for pixel range [p0, p1).

            Engine split: q-squares on ACT and ns/z on Pool for steady
            phases (DVE is the scarce engine); the final phase minimizes
            chain latency instead. rsqrt is a DVE f16 bit-trick + Newton,
            so ACT never reloads its function table.
            """
            st = state[s]
            hs = slice(p0, p1)
            n = p1 - p0
            tt, tg = [], []
            for ob in range(OB):
                if ps_direct is not None:
                    t = tp.tile([PB, n], F16, tag=f"t{ob}", name=f"t{ob}")
                    nc.vector.tensor_scalar_mul(out=t, in0=ps_direct[ob],
                                                scalar1=st["dcol"][ob])
                elif scaled_yc:
                    t = yc[ob][:, hs]
                else:
                    t = tp.tile([PB, n], F16, tag=f"t{ob}", name=f"t{ob}")
                    nc.vector.tensor_scalar_mul(out=t, in0=yc[ob][:, hs],
                                                scalar1=st["dcol"][ob])
                tt.append(t)
            q0 = np_.tile([PB, n], F16, tag="q", name="q0")
            q1 = np_.tile([PB, n], F16, tag="q", name="q1")
            if final:
                nc.vector.tensor_mul(out=q0, in0=tt[0], in1=tt[0])
                nc.vector.tensor_mul(out=q1, in0=tt[1], in1=tt[1])
            else:
                nc.scalar.activation(out=q0, in_=tt[0], func=AF.Square)
                nc.scalar.activation(out=q1, in_=tt[1], func=AF.Square)
            nsum = np_.tile([PB, n], F16, tag="nsum", name="nsum")
            if final:
                nc.vector.tensor_add(out=nsum, in0=q0, in1=q1)
            else:
                nc.gpsimd.tensor_add(out=nsum, in0=q0, in1=q1)
            # t*g16 off the critical path (before the partition reduce)
            for ob in range(OB):
                g = tp.tile([PB, n], F16, tag=f"tg{ob}", name=f"tg{ob}")
                nc.vector.tensor_scalar_mul(out=g, in0=tt[ob],
                                            scalar1=g16sb[ob])
                tg.append(g)
            nc.gpsimd.partition_all_reduce(
                nsum[:], nsum[:], PB, bass.bass_isa.ReduceOp.add
            )
            # f16 bit-trick rsqrt + 1 Newton iter, all on DVE
            I16 = mybir.dt.int16
            rt = np_.tile([PB, n], F16, tag="rt", name="rt")
            sd = rt.bitcast(I16)
            nc.vector.tensor_scalar(
                out=sd, in0=nsum.bitcast(I16), scalar1=1, scalar2=None,
                op0=ALU.logical_shift_right,
            )
            nc.vector.tensor_scalar(
                out=sd, in0=sd, scalar1=-1, scalar2=0x59BA,
                op0=ALU.mult, op1=ALU.add,
            )
            tn = np_.tile([PB, n], F16, tag="tn", name="tn")
            nc.vector.tensor_mul(out=tn, in0=rt, in1=rt)
            nc.vector.tensor_mul(out=tn, in0=tn, in1=nsum)
            nc.vector.tensor_scalar(
                out=tn, in0=tn, scalar1=-0.5, scalar2=1.5,
                op0=ALU.mult, op1=ALU.add,
            )
            nc.vector.tensor_mul(out=rt, in0=rt, in1=tn)
            o = outp.tile([PB, OB, n], BF16, tag="o", name="o")
            for ob in range(OB):
                # z overwrites tg in place (tg dead after this)
                if final and ob == 1:
                    nc.vector.tensor_mul(out=tg[ob], in0=tg[ob], in1=rt)
                else:
                    nc.gpsimd.tensor_mul(out=tg[ob], in0=tg[ob], in1=rt)
                if SIM_SILU:
                    sg = np_.tile([PB, n], F16, tag="tn", name=f"sg{ob}")
                    nc.scalar.activation(out=sg, in_=tg[ob], func=AF.Sigmoid)
                    nc.vector.tensor_mul(out=o[:, ob], in0=tg[ob], in1=sg)
                else:
                    nc.scalar.activation(out=o[:, ob], in_=tg[ob], func=AF.Silu)
            # one fused DMA for both ob halves (fewer HWDGE holds); final
            # phases issue from less-contended queues
            if final == "act":
                nc.scalar.dma_start(out=y_d[s, :, :, hs], in_=o)
            else:
                nc.sync.dma_start(out=y_d[s, :, :, hs], in_=o)

        # ---- main schedule ----
        QQ = HW // 4
        prep_front(0)
        yc0 = [ycp.tile([PB, HW], F16, tag=f"yc{ob}", name=f"yc{ob}")
               for ob in range(OB)]
        # s0: ob0 first (its weights land first), T3 deferred on the first
        # two tiles to ride out the xlo DMA.
        conv_tiles_deferred_x(0, [0, 1], 0, yc0)
        conv_tiles(0, range(2, NPT), yc0, obs=(0,))
        prep_demod(0)
        conv_tiles(0, range(0, 4), yc0, obs=(1,))
        if S > 1:
            prep_front(1)
            prep_demod(1)
        norm_phase(0, 0, HH, yc0)
        conv_tiles(0, range(4, NPT), yc0, obs=(1,))
        norm_phase(0, HH, HW, yc0)
        for s in range(1, S):
            yc = [ycp.tile([PB, HW], F16, tag=f"yc{ob}", name=f"yc{ob}")
                  for ob in range(OB)]
            if s + 1 < S:
                conv_tiles(s, range(0, 4), yc)
                prep_front(s + 1)
                prep_demod(s + 1)
                norm_phase(s, 0, HH, yc, scaled_yc=True)
                conv_tiles(s, range(4, NPT), yc)
                norm_phase(s, HH, HW, yc, scaled_yc=True)
            else:
                # last sample: spread phases so only the small final one
                # trails the conv
                conv_tiles(s, range(0, 2), yc)
                conv_tiles(s, range(2, 4), yc)
                norm_phase(s, 0, QQ, yc, scaled_yc=True)
                conv_tiles(s, range(4, 6), yc)
                norm_phase(s, QQ, HH, yc, scaled_yc=True)
                conv_tiles(s, range(6, 7), yc)
                norm_phase(s, HH, 3 * QQ, yc, scaled_yc=True)
                ps7a = [pconv.tile([PB, PT // 2], F32, tag="conv", name="c7a")
                        for _ in range(OB)]
                for ob in range(OB):
                    conv_tile(s, 7, ob, yc, half=(ps7a[ob], 0, 4))
                ps7b = [pconv.tile([PB, PT // 2], F32, tag="conv", name="c7b")
                        for _ in range(OB)]
                for ob in range(OB):
                    conv_tile(s, 7, ob, yc, half=(ps7b[ob], 4, 4))
                norm_phase(s, 3 * QQ, 7 * HW // 8, yc, scaled_yc=True)
                norm_phase(s, 7 * HW // 8, 15 * HW // 16, yc, scaled_yc=True,
                           final="act")
                norm_phase(s, 15 * HW // 16, HW, yc, scaled_yc=True,
                           final="act")
    nc.finalize()
    return nc


_NC_CACHE = {}


def _get_program():
    if "nc" not in _NC_CACHE:
        _NC_CACHE["nc"] = build_program()
    return _NC_CACHE["nc"]


def _host_prep(x, mod, kernel_mod, weights, gamma):
    assert ml_dtypes is not None, "ml_dtypes required for fp8 host prep"
    f8 = ml_dtypes.float8_e4m3
    bf = ml_dtypes.bfloat16

    x = np.asarray(x, dtype=np.float32)
    mod = np.asarray(mod, dtype=np.float32)
    kernel_mod = np.asarray(kernel_mod, dtype=np.float32)
    weights = np.asarray(weights, dtype=np.float32)
    gamma = np.asarray(gamma, dtype=np.float32)

    e = np.exp(kernel_mod - kernel_mod.max(axis=-1, keepdims=True))
    attn = (e / e.sum(axis=-1, keepdims=True)).astype(np.float32)     # [B, NK]
    modp1 = mod + 1.0

    # weights -> [NK, I, OB, KK, 128] ob-major inner, bf16; the trailing
    # 2*NVEC rider columns of (bank0, ib0) carry each core's sample-0 vecs
    w6 = weights.reshape(NK, OB, PB, C_IN, K, K)
    wT0 = np.ascontiguousarray(
        w6.transpose(0, 3, 1, 4, 5, 2).reshape(NK, IB, PB, OB * KK * PB)
    ).astype(bf)
    wT0 = np.concatenate(
        [wT0, np.zeros((NK, IB, PB, 2 * NVEC), dtype=bf)], axis=-1
    )

    # bank Gram stats over kk: [i, o], scaled by S_W^2
    wio = weights.transpose(0, 2, 1, 3, 4).reshape(NK, C_IN, C_OUT, KK)
    s00 = (wio[0] * wio[0]).sum(-1)
    s01 = (wio[0] * wio[1]).sum(-1)
    s11 = (wio[1] * wio[1]).sum(-1)
    smat = np.ascontiguousarray(
        (np.stack([s00, s01, s11]) * (S_W * S_W))
        .reshape(3, IB, PB, C_OUT).astype(np.float32)
    )
    g16 = np.ascontiguousarray(
        (gamma * np.sqrt(C_OUT)).astype(np.float32).reshape(OB, PB, 1)
    )

    # x variants: fp8 hi + fp8 residual(x16), zero-padded, per-partition pack
    x8 = x.astype(f8)
    xlo = ((x - x8.astype(np.float32)) * 16.0).astype(f8)
    xpack = np.zeros((B, 2, IB, PB, PADH, PADW), dtype=f8)
    xpack[:, 0, :, :, 1:H + 1, 1:W + 1] = x8.reshape(B, IB, PB, H, W)
    xpack[:, 1, :, :, 1:H + 1, 1:W + 1] = xlo.reshape(B, IB, PB, H, W)
    # -> [B, v, PB, ib*padh*padw] (partition-major for a single DMA per v)
    xpack = np.ascontiguousarray(
        xpack.transpose(0, 1, 3, 2, 4, 5).reshape(B, 2, PB, IB * PADH * PADW)
    )

    in_maps = []
    for c in range(N_CORES):
        vecs = np.empty((S, PB, NVEC), np.float32)
        for si in range(S):
            b = c * S + si
            a0, a1 = attn[b, 0], attn[b, 1]
            mp = modp1[b].reshape(IB, PB)
            m2b = (modp1[b] * modp1[b]).reshape(IB, PB)
            for ib in range(IB):
                vecs[si, :, ib] = a0 * mp[ib]
                vecs[si, :, IB + ib] = a1 * mp[ib]
                vecs[si, :, 2 * IB + 3 * ib + 0] = m2b[ib] * (a0 * a0)
                vecs[si, :, 2 * IB + 3 * ib + 1] = m2b[ib] * (2.0 * a0 * a1)
                vecs[si, :, 2 * IB + 3 * ib + 2] = m2b[ib] * (a1 * a1)
        sl = slice(c * S, (c + 1) * S)
        wTc = wT0.copy()
        wTc[0, 0, :, KK * C_OUT:] = (
            np.ascontiguousarray(vecs[0]).view(np.uint16).view(bf)
        )
        in_maps.append({
            "x8": xpack[sl],
            "wT": wTc,
            "smat": smat,
            "vecs": vecs,
            "g16": g16,
        })
    return in_maps


def kernel(x, mod, kernel_mod, weights, gamma, _trace=False, _trace_kwargs=None):
    nc = _get_program()
    in_maps = _host_prep(x, mod, kernel_mod, weights, gamma)
    res = run_bass_kernel_spmd(
        nc, in_maps, list(range(N_CORES)),
        trace=_trace, **(_trace_kwargs or {}),
    )
    y = np.concatenate(
        [np.asarray(res.results[c]["y"]).astype(np.float32)
         .reshape(S, PB, OB, HW).transpose(0, 2, 1, 3)
         .reshape(S, C_OUT, H, W) for c in range(N_CORES)],
        axis=0,
    )
    if _trace:
        kernel.last_results = res
    return y


kernel.last_results = None


# revision 42
# speedup vs baseline: 1.5567x; 1.0048x over previous
"""Trainium2 Bass kernel for nn_Block_40742059770386 (dense_cnn).

Per-sample adaptively-mixed, style-modulated, demodulated 3x3 conv
(StyleGAN2-style) + channel RMS norm + SiLU.

Sharding: data-parallel over batch. B=16 samples -> 8 cores x 2 samples.
The kernel bank (2 x 256 x 256 x 3 x 3, shipped bf16) and gamma are
replicated.

Conv engine strategy (the PE-time floor):
  fp8e4m3 DoubleRow matmuls contract all 256 input channels per
  instruction at 0.5 cycles/row -- 4x the f32r row rate.  Raw fp8 on
  both operands misses the 2e-2 tolerance (~4e-2), so the conv runs a
  3-term hi/lo split at a shared PSUM scale S=128:

      S*w*x ~= w8@x8 + wlo16@x8 + w8_16@xlo8
      w8    = fp8(S*wm)            (wm = bf16-mixed modulated weights)
      wlo16 = fp8(S*wm - w8)       (weight residual, same S scale)
      w8_16 = fp8(S/16*wm)         (for the x-residual term)
      x8    = fp8(x), xlo8 = fp8(16*(x - x8))   (host-side dtype prep)

  27 DoubleRow matmuls per [128 out, 512 px] tile vs 18 f32r ones:
  1.33x less PE time at ~bf16 accuracy (measured 4.6e-3 end to end).

Modulation is folded into the mix coefficients (a_n * (mod+1)[ci] as
per-partition scalar columns), so x ships unmodulated and pre-padded;
no on-chip modulate/pad pass at all.

Epilogue: Pool drains PSUM -> f16 (427ns/tile), everything else in f16
on DVE at 2x/4x rates; channel-sum via gpsimd partition_all_reduce
(fp32 internal), rsqrt = ACT Sqrt + DVE reciprocal, SiLU on ACT,
y ships bf16.  Demod d = rsqrt(Gram-matvec) kept fp32 as before.
"""

import os

import numpy as np

import concourse.bass as bass
import concourse.bacc as bacc
import concourse.mybir as mybir
import concourse.tile as tile
from contextlib import ExitStack
from concourse.bass_utils import run_bass_kernel_spmd

try:
    import ml_dtypes
except ImportError:  # pragma: no cover
    ml_dtypes = None

# ---- problem constants (hardcoded; kernel.py must be self-contained) ----
B, C_IN, C_OUT, H, W, K, NK = 16, 256, 256, 64, 64, 3, 2
N_CORES = 8
S = B // N_CORES            # samples per core
PB = 128                    # partitions per block
IB = C_IN // PB             # input channel blocks (= DoubleRow k-tiles)
OB = C_OUT // PB            # output channel blocks
HW = H * W                  # 4096
PADH, PADW = H + 2, W + 2   # 66, 66
PT = 512                    # pixels per tile (one PSUM bank of fp32)
ROWS_PT = PT // W           # 8 rows per pixel tile
NPT = HW // PT              # 8 pixel tiles
KK = K * K                  # 9
NVEC = 2 * IB + 3 * IB  # a0m/a1m per ib, m2k per ib
EPS = 1e-8
S_W = 128.0                 # PSUM scale for the fp8 hi/lo split
HH = HW // 2                # half-sample pixels (norm phase granularity)

F32 = mybir.dt.float32
F16 = mybir.dt.float16
BF16 = mybir.dt.bfloat16
F8 = mybir.dt.float8e4
I32 = mybir.dt.int32

AF = mybir.ActivationFunctionType
ALU = mybir.AluOpType
PM = mybir.MatmulPerfMode
MAGIC = 0x5F3759DF
# CoreSim does not implement Silu; decompose for sim-only runs
SIM_SILU = os.environ.get("KERNEL_SIM_SILU", "0") == "1"
# CoreSim's DoubleRow exec path rejects 4-dim moving APs; emit equivalent
# per-ib fp8 matmuls for sim-only functional checks
SIM_NO_DR = os.environ.get("KERNEL_SIM_NO_DR", "0") == "1"


def _rsqrt_dve(nc, pool, src_ap, clamp, shape, tag, iters=2):
    """rsqrt(max(src, clamp)) on DVE: bit-trick seed + Newton (fp32)."""
    x = pool.tile(shape, F32, tag=f"{tag}_x", name=f"{tag}_x")
    nc.vector.tensor_scalar_max(out=x, in0=src_ap, scalar1=float(clamp))
    seed = pool.tile(shape, I32, tag=f"{tag}_s", name=f"{tag}_s")
    nc.vector.tensor_scalar(
        out=seed, in0=x.bitcast(I32), scalar1=1, scalar2=None,
        op0=ALU.logical_shift_right,
    )
    nc.vector.tensor_scalar(
        out=seed, in0=seed, scalar1=-1, scalar2=MAGIC,
        op0=ALU.mult, op1=ALU.add,
    )
    r = seed.bitcast(F32)
    xh = pool.tile(shape, F32, tag=f"{tag}_xh", name=f"{tag}_xh")
    nc.vector.tensor_scalar_mul(out=xh, in0=x, scalar1=0.5)
    for it in range(iters):
        t = pool.tile(shape, F32, tag=f"{tag}_t{it}", name=f"{tag}_t{it}")
        nc.vector.tensor_mul(out=t, in0=r, in1=r)
        nc.vector.tensor_mul(out=t, in0=t, in1=xh)
        nc.vector.tensor_scalar(
            out=t, in0=t, scalar1=-1.0, scalar2=1.5, op0=ALU.mult, op1=ALU.add
        )
        r2 = pool.tile(shape, F32, tag=f"{tag}_r{it}", name=f"{tag}_r{it}")
        nc.vector.tensor_mul(out=r2, in0=r, in1=t)
        r = r2
    return r


def build_program():
    nc = bacc.Bacc(trn_type="TRN2", debug=False)

    # x pack: [sample, variant(x8, xlo8), partition, ib*padh*padw] fp8
    x_d = nc.declare_dram_parameter("x8", [S, 2, PB, IB * PADH * PADW], F8,
                                    isOutput=False)
    # weight banks: [bank, ib, partition, ob*kk*128] bf16
    wt_d = nc.declare_dram_parameter("wT", [NK, IB, PB, KK * C_OUT], BF16,
                                     isOutput=False)
    vecs_d = nc.declare_dram_parameter("vecs", [S, PB, NVEC], F32, isOutput=False)
    smat_d = nc.declare_dram_parameter("smat", [3, IB, PB, C_OUT], F32,
                                       isOutput=False)
    g16_d = nc.declare_dram_parameter("g16", [OB, PB, 1], F32, isOutput=False)
    y_d = nc.declare_dram_parameter("y", [S, PB, OB, HW], BF16, isOutput=True)

    with ExitStack() as ctx:
        tc = ctx.enter_context(tile.TileContext(nc))
        const = ctx.enter_context(tc.tile_pool(name="const", bufs=1))
        small = ctx.enter_context(tc.tile_pool(name="small", bufs=4))
        wch = ctx.enter_context(tc.tile_pool(name="wch", bufs=1))
        wq = ctx.enter_context(tc.tile_pool(name="wq", bufs=2))
        xq = ctx.enter_context(tc.tile_pool(name="xq", bufs=2))
        ycp = ctx.enter_context(tc.tile_pool(name="ycp", bufs=1))
        tp = ctx.enter_context(tc.tile_pool(name="tp", bufs=2))
        np_ = ctx.enter_context(tc.tile_pool(name="normp", bufs=2))
        outp = ctx.enter_context(tc.tile_pool(name="outp", bufs=2))
        pconv = ctx.enter_context(tc.tile_pool(name="pconv", bufs=6, space="PSUM"))
        pdsq = ctx.enter_context(tc.tile_pool(name="pdsq", bufs=1, space="PSUM"))

        # ---- resident constants ----
        wbank = [
            [const.tile([PB, KK * C_OUT], BF16, tag=f"wb{n}{ib}", name=f"wb{n}{ib}")
             for ib in range(IB)]
            for n in range(NK)
        ]
        g16sb = [const.tile([PB, 1], F32, tag=f"g16_{ob}", name=f"g16_{ob}")
                 for ob in range(OB)]
        smat = [
            [const.tile([PB, C_OUT], F32, tag=f"sm{k}{ib}", name=f"sm{k}{ib}")
             for ib in range(IB)]
            for k in range(3)
        ]
        # scalar constants for ACT scale/bias (must be APs)
        c_sw = const.tile([PB, 1], F32, tag="c_sw", name="c_sw")
        nc.vector.memset(c_sw, S_W)
        c_sw16 = const.tile([PB, 1], F32, tag="c_sw16", name="c_sw16")
        nc.vector.memset(c_sw16, S_W / 16.0)
        c_nrm = const.tile([PB, 1], F32, tag="c_nrm", name="c_nrm")
        nc.vector.memset(c_nrm, 1e-6)
        # PE p-state warm-up: ~6.5us of dummy matmuls during the startup
        # DMA window so the first conv tile runs at the full 2.4GHz clock
        dmy = const.tile([PB, 64], F32, tag="dmy", name="dmy")
        nc.vector.memset(dmy, 0.0)
        pwarm = ctx.enter_context(tc.tile_pool(name="pwarm", bufs=1,
                                               space="PSUM"))
        wps = pwarm.tile([1, 64], F32, tag="wps", name="wps")
        for _ in range(40):
            nc.tensor.matmul(wps, lhsT=c_nrm, rhs=dmy, start=True, stop=True)

        # per-sample state produced by the prep phase, consumed by conv/norm
        state = {}

        def prep_front(s):
            """DMAs + weight chain for sample s (no PE work)."""
            vec = small.tile([PB, NVEC], F32, tag="vec", name="vec")
            if s == 0:
                # off the SP queue so the weight DMAs launch immediately
                nc.gpsimd.dma_start(out=vec, in_=vecs_d[s])
            else:
                nc.sync.dma_start(out=vec, in_=vecs_d[s])
            a0m = [vec[:, ib:ib + 1] for ib in range(IB)]
            a1m = [vec[:, IB + ib:IB + ib + 1] for ib in range(IB)]
            m2k = [[vec[:, 2 * IB + 3 * ib + k:2 * IB + 3 * ib + k + 1]
                    for k in range(3)] for ib in range(IB)]

            HOB = KK * PB  # 1152 inner columns per ob half
            if s == 0:
                for ib in range(IB):
                    for n in range(NK):
                        nc.sync.dma_start(out=wbank[n][ib][:, 0:HOB],
                                          in_=wt_d[n, ib, :, 0:HOB])
            # x pack for this sample: [128, v, ib, padh, padw]
            xp = xq.tile([PB, 2, IB, PADH, PADW], F8, tag="xp", name="xp")
            for v in range(2):
                xsrc = x_d[s, v].rearrange("p (i h w) -> p i h w", i=IB, w=PADW)
                if s == 0 and v == 0:
                    # rows 0..17 cover pixel tiles 0-1: start conv sooner
                    nc.sync.dma_start(out=xp[:, v, :, 0:18], in_=xsrc[:, :, 0:18])
                    nc.sync.dma_start(out=xp[:, v, :, 18:PADH], in_=xsrc[:, :, 18:PADH])
                else:
                    nc.sync.dma_start(out=xp[:, v], in_=xsrc)
                if s == 0 and v == 1:
                    for ib in range(IB):
                        for n in range(NK):
                            nc.sync.dma_start(out=wbank[n][ib][:, HOB:2 * HOB],
                                              in_=wt_d[n, ib, :, HOB:2 * HOB])
            if s == 0:
                for k in range(3):
                    for ib in range(IB):
                        nc.sync.dma_start(out=smat[k][ib], in_=smat_d[k, ib])
                for ob in range(OB):
                    nc.sync.dma_start(out=g16sb[ob], in_=g16_d[ob])

            # ---- weight chain: bf16 mix -> fp8 {w8, wlo16, w8_16} ----
            # w8 first (gates the conv start), residuals after.  On the
            # first sample, run per ob-half so conv starts off half the DMA.
            w8 = wq.tile([PB, IB, KK * C_OUT], F8, tag="w8", name="w8")
            wlo = wq.tile([PB, IB, KK * C_OUT], F8, tag="wlo", name="wlo")
            w816 = wq.tile([PB, IB, KK * C_OUT], F8, tag="w816", name="w816")
            cols = [slice(ob * HOB, (ob + 1) * HOB) for ob in range(OB)] \
                if s == 0 else [slice(0, OB * HOB)]
            wms = {}
            for cs in cols:
                for ib in range(IB):
                    m1 = wch.tile([PB, KK * C_OUT], BF16, tag="wa", name=f"m1_{ib}")
                    nc.vector.tensor_scalar_mul(out=m1[:, cs],
                                                in0=wbank[0][ib][:, cs],
                                                scalar1=a0m[ib])
                    m2 = wch.tile([PB, KK * C_OUT], BF16, tag="wb", name=f"m2_{ib}")
                    nc.vector.tensor_scalar_mul(out=m2[:, cs],
                                                in0=wbank[1][ib][:, cs],
                                                scalar1=a1m[ib])
                    wm = wch.tile([PB, KK * C_OUT], BF16, tag=f"wc{ib}",
                                  name=f"wm_{ib}")
                    nc.vector.tensor_add(out=wm[:, cs], in0=m1[:, cs],
                                         in1=m2[:, cs])
                    if s == 0 and cs.start == 0 and ib == 1:
                        # DVE quant overlaps ACT's ib0 quant: earlier conv start
                        nc.vector.tensor_scalar_mul(out=w8[:, ib, cs],
                                                    in0=wm[:, cs], scalar1=S_W)
                    else:
                        nc.scalar.activation(out=w8[:, ib, cs], in_=wm[:, cs],
                                             func=AF.Copy, scale=c_sw)
                    wms[ib] = wm
                for ib in range(IB):
                    wm = wms[ib]
                    nc.scalar.activation(out=w816[:, ib, cs], in_=wm[:, cs],
                                         func=AF.Copy, scale=c_sw16)
                    w8n = wch.tile([PB, KK * C_OUT], BF16, tag="wa",
                                   name=f"w8n_{ib}")
                    nc.vector.tensor_scalar_mul(out=w8n[:, cs], in0=w8[:, ib, cs],
                                                scalar1=-1.0 / S_W)
                    rres = wch.tile([PB, KK * C_OUT], BF16, tag="wb",
                                    name=f"rr_{ib}")
                    nc.vector.tensor_add(out=rres[:, cs], in0=wm[:, cs],
                                         in1=w8n[:, cs])
                    nc.vector.tensor_scalar_mul(out=wlo[:, ib, cs],
                                                in0=rres[:, cs], scalar1=S_W)
            state[s] = dict(xp=xp, w8=w8, wlo=wlo, w816=w816, m2k=m2k)

        def prep_demod(s):
            """dsq Gram matvecs (PE) + d' rsqrt (DVE) for sample s."""
            m2k = state[s]["m2k"]
            dsq_ps = pdsq.tile([PB, OB], F32, tag="dsq", name="dsq")
            for ob in range(OB):
                i_mv = 0
                for ib in range(IB):
                    for k in range(3):
                        nc.tensor.matmul(
                            dsq_ps[:, ob:ob + 1],
                            lhsT=smat[k][ib][:, ob * PB:(ob + 1) * PB],
                            rhs=m2k[ib][k],
                            start=(i_mv == 0),
                            stop=(i_mv == 3 * IB - 1),
                        )
                        i_mv += 1
            dcol = []
            for ob in range(OB):
                d = _rsqrt_dve(nc, small, dsq_ps[:, ob:ob + 1],
                               EPS * S_W * S_W, [PB, 1], f"d{ob}")
                dcol.append(d)
            state[s]["dcol"] = dcol

        def emit_group(s, pt, ob, ps, terms, start, stop, half=None):
            st = state[s]
            xp = st["xp"]
            r0, nr = (0, ROWS_PT) if half is None else (half[1], half[2])
            n_mm = len(terms) * KK * (IB if SIM_NO_DR else 1)
            i_mm = 0
            for wt_name, v in terms:
                wt = st[wt_name]
                for ki in range(K):
                    for kj in range(K):
                        kk = ki * K + kj
                        pso = ps if half is None else ps[:, 0:nr * W]
                        if SIM_NO_DR:
                            for ib in range(IB):
                                nc.tensor.matmul(
                                    pso,
                                    lhsT=wt[:, ib, (ob * KK + kk) * PB:
                                            (ob * KK + kk) * PB + PB],
                                    rhs=xp[:, v, ib,
                                           pt * ROWS_PT + r0 + ki:
                                           pt * ROWS_PT + r0 + ki + nr,
                                           kj: kj + W],
                                    start=(start and i_mm == 0),
                                    stop=(stop and i_mm == n_mm - 1),
                                )
                                i_mm += 1
                            continue
                        lhsT = wt[:, :, (ob * KK + kk) * PB:
                                  (ob * KK + kk) * PB + PB]
                        rhs = xp[:, v, :,
                                 pt * ROWS_PT + r0 + ki:
                                 pt * ROWS_PT + r0 + ki + nr,
                                 kj: kj + W]
                        nc.tensor.matmul(
                            pso, lhsT=lhsT, rhs=rhs,
                            start=(start and i_mm == 0),
                            stop=(stop and i_mm == n_mm - 1),
                            perf_mode=PM.DoubleRow,
                        )
                        i_mm += 1

        T_MAIN = [("w8", 0), ("wlo", 0)]
        T_X = [("w816", 1)]

        def conv_tile(s, pt, ob, yc, drain=True, half=None):
            ps = pconv.tile([PB, PT], F32, tag="conv", name="conv") \
                if half is None else half[0]
            emit_group(s, pt, ob, ps, T_MAIN + T_X, True, True, half=half)
            if drain:
                # fold the demod scale into the drain for s>0 (sample 0's
                # dcol is not ready when its early tiles drain; its phases
                # apply the scale instead)
                sc = state[s]["dcol"][ob] if s > 0 else 1.0
                p0 = pt * PT if half is None else pt * PT + half[1] * W
                n = PT if half is None else half[2] * W
                nc.scalar.activation(out=yc[ob][:, p0:p0 + n],
                                     in_=ps[:, 0:n], func=AF.Copy, scale=sc)
            return ps

        def conv_tiles(s, pt_range, yc, obs=(0, 1)):
            for pt in pt_range:
                for ob in obs:
                    conv_tile(s, pt, ob, yc)

        def conv_tiles_deferred_x(s, pts, ob, yc):
            """T1+T2 of each tile first, T3 after: hides the xlo DMA."""
            pss = {}
            for pt in pts:
                pss[pt] = pconv.tile([PB, PT], F32, tag="conv", name="conv")
                emit_group(s, pt, ob, pss[pt], T_MAIN, True, False)
            for pt in pts:
                emit_group(s, pt, ob, pss[pt], T_X, False, True)
                nc.scalar.activation(out=yc[ob][:, pt * PT:(pt + 1) * PT],
                                     in_=pss[pt], func=AF.Copy)

        def norm_phase(s, p0, p1, yc, ps_direct=None, scaled_yc=False,
                       final=False):
            """RMS-norm + SiLU + store for pixel range [p0, p1).

            Engine split: q-squares on ACT and ns/z on Pool for steady
            phases (DVE is the scarce engine); the final phase minimizes
            chain latency instead. rsqrt is a DVE f16 bit-trick + Newton,
            so ACT never reloads its function table.
            """
            st = state[s]
            hs = slice(p0, p1)
            n = p1 - p0
            tt, tg = [], []
            for ob in range(OB):
                if ps_direct is not None:
                    t = tp.tile([PB, n], F16, tag=f"t{ob}", name=f"t{ob}")
                    nc.vector.tensor_scalar_mul(out=t, in0=ps_direct[ob],
                                                scalar1=st["dcol"][ob])
                elif scaled_yc:
                    t = yc[ob][:, hs]
                else:
                    t = tp.tile([PB, n], F16, tag=f"t{ob}", name=f"t{ob}")
                    nc.vector.tensor_scalar_mul(out=t, in0=yc[ob][:, hs],
                                                scalar1=st["dcol"][ob])
                tt.append(t)
            q0 = np_.tile([PB, n], F16, tag="q", name="q0")
            q1 = np_.tile([PB, n], F16, tag="q", name="q1")
            if final:
                nc.vector.tensor_mul(out=q0, in0=tt[0], in1=tt[0])
                nc.vector.tensor_mul(out=q1, in0=tt[1], in1=tt[1])
            else:
                nc.scalar.activation(out=q0, in_=tt[0], func=AF.Square)
                nc.scalar.activation(out=q1, in_=tt[1], func=AF.Square)
            nsum = np_.tile([PB, n], F16, tag="nsum", name="nsum")
            if final:
                nc.vector.tensor_add(out=nsum, in0=q0, in1=q1)
            else:
                nc.gpsimd.tensor_add(out=nsum, in0=q0, in1=q1)
            # t*g16 off the critical path (before the partition reduce)
            for ob in range(OB):
                g = tp.tile([PB, n], F16, tag=f"tg{ob}", name=f"tg{ob}")
                nc.vector.tensor_scalar_mul(out=g, in0=tt[ob],
                                            scalar1=g16sb[ob])
                tg.append(g)
            nc.gpsimd.partition_all_reduce(
                nsum[:], nsum[:], PB, bass.bass_isa.ReduceOp.add
            )
            # f16 bit-trick rsqrt + 1 Newton iter, all on DVE
            I16 = mybir.dt.int16
            rt = np_.tile([PB, n], F16, tag="rt", name="rt")
            sd = rt.bitcast(I16)
            nc.vector.tensor_scalar(
                out=sd, in0=nsum.bitcast(I16), scalar1=1, scalar2=None,
                op0=ALU.logical_shift_right,
            )
            nc.vector.tensor_scalar(
                out=sd, in0=sd, scalar1=-1, scalar2=0x59BA,
                op0=ALU.mult, op1=ALU.add,
            )
            tn = np_.tile([PB, n], F16, tag="tn", name="tn")
            nc.vector.tensor_mul(out=tn, in0=rt, in1=rt)
            nc.vector.tensor_mul(out=tn, in0=tn, in1=nsum)
            nc.vector.tensor_scalar(
                out=tn, in0=tn, scalar1=-0.5, scalar2=1.5,
                op0=ALU.mult, op1=ALU.add,
            )
            nc.vector.tensor_mul(out=rt, in0=rt, in1=tn)
            o = outp.tile([PB, OB, n], BF16, tag="o", name="o")
            for ob in range(OB):
                # z overwrites tg in place (tg dead after this)
                if final and ob == 1:
                    nc.vector.tensor_mul(out=tg[ob], in0=tg[ob], in1=rt)
                else:
                    nc.gpsimd.tensor_mul(out=tg[ob], in0=tg[ob], in1=rt)
                if SIM_SILU:
                    sg = np_.tile([PB, n], F16, tag="tn", name=f"sg{ob}")
                    nc.scalar.activation(out=sg, in_=tg[ob], func=AF.Sigmoid)
                    nc.vector.tensor_mul(out=o[:, ob], in0=tg[ob], in1=sg)
                else:
                    nc.scalar.activation(out=o[:, ob], in_=tg[ob], func=AF.Silu)
            # one fused DMA for both ob halves (fewer HWDGE holds); final
            # phases issue from less-contended queues
            if final == "act":
                nc.scalar.dma_start(out=y_d[s, :, :, hs], in_=o)
            else:
                nc.sync.dma_start(out=y_d[s, :, :, hs], in_=o)

        # ---- main schedule ----
        QQ = HW // 4
        prep_front(0)
        yc0 = [ycp.tile([PB, HW], F16, tag=f"yc{ob}", name=f"yc{ob}")
               for ob in range(OB)]
        # s0: ob0 first (its weights land first), T3 deferred on the first
        # two tiles to ride out the xlo DMA.
        conv_tiles_deferred_x(0, [0, 1], 0, yc0)
        conv_tiles(0, range(2, NPT), yc0, obs=(0,))
        prep_demod(0)
        conv_tiles(0, range(0, 4), yc0, obs=(1,))
        if S > 1:
            prep_front(1)
            prep_demod(1)
        norm_phase(0, 0, HH, yc0)
        conv_tiles(0, range(4, NPT), yc0, obs=(1,))
        norm_phase(0, HH, HW, yc0)
        for s in range(1, S):
            yc = [ycp.tile([PB, HW], F16, tag=f"yc{ob}", name=f"yc{ob}")
                  for ob in range(OB)]
            if s + 1 < S:
                conv_tiles(s, range(0, 4), yc)
                prep_front(s + 1)
                prep_demod(s + 1)
                norm_phase(s, 0, HH, yc, scaled_yc=True)
                conv_tiles(s, range(4, NPT), yc)
                norm_phase(s, HH, HW, yc, scaled_yc=True)
            else:
                # last sample: spread phases so only the small final one
                # trails the conv
                conv_tiles(s, range(0, 2), yc)
                conv_tiles(s, range(2, 4), yc)
                norm_phase(s, 0, QQ, yc, scaled_yc=True)
                conv_tiles(s, range(4, 6), yc)
                norm_phase(s, QQ, HH, yc, scaled_yc=True)
                conv_tiles(s, range(6, 7), yc)
                norm_phase(s, HH, 3 * QQ, yc, scaled_yc=True)
                ps7a = [pconv.tile([PB, PT // 2], F32, tag="conv", name="c7a")
                        for _ in range(OB)]
                for ob in range(OB):
                    conv_tile(s, 7, ob, yc, half=(ps7a[ob], 0, 4))
                ps7b = [pconv.tile([PB, PT // 2], F32, tag="conv", name="c7b")
                        for _ in range(OB)]
                for ob in range(OB):
                    conv_tile(s, 7, ob, yc, half=(ps7b[ob], 4, 4))
                norm_phase(s, 3 * QQ, 7 * HW // 8, yc, scaled_yc=True)
                norm_phase(s, 7 * HW // 8, 15 * HW // 16, yc, scaled_yc=True,
                           final="act")
                norm_phase(s, 15 * HW // 16, HW, yc, scaled_yc=True,
                           final="act")
    nc.finalize()
    return nc


_NC_CACHE = {}


def _get_program():
    if "nc" not in _NC_CACHE:
        _NC_CACHE["nc"] = build_program()
    return _NC_CACHE["nc"]


def _host_prep(x, mod, kernel_mod, weights, gamma):
    assert ml_dtypes is not None, "ml_dtypes required for fp8 host prep"
    f8 = ml_dtypes.float8_e4m3
    bf = ml_dtypes.bfloat16

    x = np.asarray(x, dtype=np.float32)
    mod = np.asarray(mod, dtype=np.float32)
    kernel_mod = np.asarray(kernel_mod, dtype=np.float32)
    weights = np.asarray(weights, dtype=np.float32)
    gamma = np.asarray(gamma, dtype=np.float32)

    e = np.exp(kernel_mod - kernel_mod.max(axis=-1, keepdims=True))
    attn = (e / e.sum(axis=-1, keepdims=True)).astype(np.float32)     # [B, NK]
    modp1 = mod + 1.0

    # weights -> [NK, I, OB, KK, 128] ob-major inner, bf16
    w6 = weights.reshape(NK, OB, PB, C_IN, K, K)
    wT = np.ascontiguousarray(
        w6.transpose(0, 3, 1, 4, 5, 2).reshape(NK, IB, PB, OB * KK * PB)
    ).astype(bf)

    # bank Gram stats over kk: [i, o], scaled by S_W^2
    wio = weights.transpose(0, 2, 1, 3, 4).reshape(NK, C_IN, C_OUT, KK)
    s00 = (wio[0] * wio[0]).sum(-1)
    s01 = (wio[0] * wio[1]).sum(-1)
    s11 = (wio[1] * wio[1]).sum(-1)
    smat = np.ascontiguousarray(
        (np.stack([s00, s01, s11]) * (S_W * S_W))
        .reshape(3, IB, PB, C_OUT).astype(np.float32)
    )
    g16 = np.ascontiguousarray(
        (gamma * np.sqrt(C_OUT)).astype(np.float32).reshape(OB, PB, 1)
    )

    # x variants: fp8 hi + fp8 residual(x16), zero-padded, per-partition pack
    x8 = x.astype(f8)
    xlo = ((x - x8.astype(np.float32)) * 16.0).astype(f8)
    xpack = np.zeros((B, 2, IB, PB, PADH, PADW), dtype=f8)
    xpack[:, 0, :, :, 1:H + 1, 1:W + 1] = x8.reshape(B, IB, PB, H, W)
    xpack[:, 1, :, :, 1:H + 1, 1:W + 1] = xlo.reshape(B, IB, PB, H, W)
    # -> [B, v, PB, ib*padh*padw] (partition-major for a single DMA per v)
    xpack = np.ascontiguousarray(
        xpack.transpose(0, 1, 3, 2, 4, 5).reshape(B, 2, PB, IB * PADH * PADW)
    )

    in_maps = []
    for c in range(N_CORES):
        vecs = np.empty((S, PB, NVEC), np.float32)
        for si in range(S):
            b = c * S + si
            a0, a1 = attn[b, 0], attn[b, 1]
            mp = modp1[b].reshape(IB, PB)
            m2b = (modp1[b] * modp1[b]).reshape(IB, PB)
            for ib in range(IB):
                vecs[si, :, ib] = a0 * mp[ib]
                vecs[si, :, IB + ib] = a1 * mp[ib]
                vecs[si, :, 2 * IB + 3 * ib + 0] = m2b[ib] * (a0 * a0)
                vecs[si, :, 2 * IB + 3 * ib + 1] = m2b[ib] * (2.0 * a0 * a1)
                vecs[si, :, 2 * IB + 3 * ib + 2] = m2b[ib] * (a1 * a1)
        sl = slice(c * S, (c + 1) * S)
        in_maps.append({
            "x8": xpack[sl],
            "wT": wT,
            "smat": smat,
            "vecs": vecs,
            "g16": g16,
        })
    return in_maps


def kernel(x, mod, kernel_mod, weights, gamma, _trace=False, _trace_kwargs=None):
    nc = _get_program()
    in_maps = _host_prep(x, mod, kernel_mod, weights, gamma)
    res = run_bass_kernel_spmd(
        nc, in_maps, list(range(N_CORES)),
        trace=_trace, **(_trace_kwargs or {}),
    )
    y = np.concatenate(
        [np.asarray(res.results[c]["y"]).astype(np.float32)
         .reshape(S, PB, OB, HW).transpose(0, 2, 1, 3)
         .reshape(S, C_OUT, H, W) for c in range(N_CORES)],
        axis=0,
    )
    if _trace:
        kernel.last_results = res
    return y


kernel.last_results = None
